# revision 18
# baseline (speedup 1.0000x reference)
"""Trainium2 Bass kernel for CheemsNonWoAttn (GQA attention block, no Wo).

Sharding: 8 cores = batch(2) x kv-head-pair(4). Each core handles one batch
element and 2 of the 8 kv heads (GQA: Q and K are repeated identically across
the 4 groups, so only 8 unique softmax matrices exist; V uses all 32 heads).

Per-core device program (v2, stall-free schedule):
  warmup MMs during DMA lead-in (HAM clock-gate release)
  Qt/Kt = Wq^T X^T, Wk^T X^T    (d on partitions; N=512 matmuls, K accum=16)
  psum evac split: ACT copies Q psums, DVE copies K psums -> banks free fast
  RoPE on DVE in fp16 (2x rate), 6 ops per 1024-col pair, off critical path
  V = X Wv interleaved with attention work (pump), scores gated so the
  in-order PE queue never waits on rope
  scores^T[k, q] = Kt^T Qt      (per head, K=64)
  E = exp(scores) on ACT        (no max subtraction: |scores| <~ 6, safe)
  causal mask: multiply 128x128 diagonal blocks by triangular mask (DVE)
  out = E^T V with a ones-column in V producing the softmax denominator
  out /= denom (DVE reciprocal + tensor_scalar), fp16 out, host upconverts
"""
import os
import sys
import types
from contextlib import ExitStack

for _p in ("/opt/trn_rl_repo", "/root/.axon_site/_ro/trn_rl_repo"):
    if os.path.isdir(_p) and _p not in sys.path:
        sys.path.append(_p)

import numpy as np

import concourse.bass as bass
import concourse.tile as tile
from concourse import mybir
from concourse.bass_utils import run_bass_kernel_spmd
from concourse.vector_clock import ScopedClock

# ---------------------------------------------------------------------------
# Patch 1: walrus rejects Drain instructions with >1 sync wait (CTRL ops have
# a single wait slot). Split the TileContext exit drain's waits across extra
# SP nops, one wait each.
def _patched_drain_and_barrier(self, tick_clock, wait_clock):
    nc = self.nc
    drain_bi = nc.sync.drain()
    wait_clock.add_sem_waits(drain_bi.ins, ScopedClock({None: tick_clock.global_clock}))
    inst = drain_bi.ins
    si = inst.sync_info
    if si is not None and si.on_wait is not None and len(si.on_wait) > 1:
        waits = list(si.on_wait)
        inst.sync_info = mybir.SyncInfo(
            on_wait=waits[:1],
            on_update=list(si.on_update) if si.on_update else [],
        )
        for w in waits[1:]:
            nbi = nc.sync.nop()
            nbi.ins.sync_info = mybir.SyncInfo(on_wait=[w], on_update=[])
    nc.all_engine_barrier()
    assert self.sems is not None
    popped = nc._tile_sem_poison_stack.pop()
    assert popped is self._sem_poison
    nc.clear_and_free_semaphores(list(self.sems.allocated().values()))
    if not os.environ.get("CHEEMS_TRIM_EXIT_BARRIER"):
        nc.all_engine_barrier()


tile.TileContext._drain_and_barrier = _patched_drain_and_barrier


def _legalize_waits(nc):
    """This walrus build accepts at most one sync-wait per instruction.
    Split any instruction carrying N>1 waits into N-1 preceding same-engine
    nops (engines are in-order, so semantics are preserved)."""
    uid = 0
    for f in nc.m.functions:
        for blk in f.blocks:
            insts = list(blk.instructions)
            out, changed = [], False
            for inst in insts:
                si = getattr(inst, "sync_info", None)
                if si is not None and si.on_wait is not None and len(si.on_wait) > 1:
                    waits = list(si.on_wait)
                    for w in waits[:-1]:
                        uid += 1
                        out.append(mybir.InstNoOp(
                            name=f"{inst.name}_lw{uid}",
                            engine=inst.engine,
                            sync_info=mybir.SyncInfo(on_wait=[w], on_update=[]),
                            bass_nofuse=True,
                        ))
                    inst.sync_info = mybir.SyncInfo(
                        on_wait=waits[-1:],
                        on_update=list(si.on_update) if si.on_update else [],
                    )
                    changed = True
                out.append(inst)
            if changed:
                blk.instructions = out


def _dedup_ldweights(nc):
    """Clear the self-load on matmuls whose stationary operand AP is identical
    to the immediately preceding matmul in the final PE stream (walrus runs
    with ldw-opt disabled, so it reloads weights for every matmul otherwise)."""
    if os.environ.get("CHEEMS_NO_LDW_DEDUP"):
        return
    for f in nc.m.functions:
        for blk in f.blocks:
            prev_key = None
            for inst in blk.instructions:
                if not isinstance(inst, mybir.InstMatmult):
                    if isinstance(inst, mybir.InstLdweights):
                        prev_key = None
                    continue
                key = (repr(inst.ins[1]), inst.perf_mode, inst.is_transpose,
                       repr(inst.tile_position))
                if prev_key is not None and key == prev_key:
                    inst.ldweights = False
                prev_key = key


# Patch 2 (optional, for tracing): recreate the antenv.axon_hooks shim so
# run_bass_kernel_spmd(trace=True) can capture NTFF profiles under axon.
def _install_ntff_hook():
    try:
        if "antenv.axon_hooks" in sys.modules:
            return
        import antenv
        from trn_agent_boot.trn_boot import _ntff_profile_via_ctypes

        hook = _ntff_profile_via_ctypes("/opt/axon/libaxon_pjrt.so")
        mod = types.ModuleType("antenv.axon_hooks")
        mod._hook = hook
        mod.get_axon_ntff_profile_hook = lambda: mod._hook

        def _set(h):
            mod._hook = h

        mod.set_axon_ntff_profile_hook = _set
        sys.modules["antenv.axon_hooks"] = mod
        antenv.axon_hooks = mod
    except Exception:
        pass


# ---------------------------------------------------------------------------
B, S, HID = 2, 2048, 2048
NH, G = 32, 4
HD = 64          # head dim
HKV = 8          # kv heads
THETA = 10000.0
P = 128          # partitions
NKT = HID // P   # 16 k-tiles over the contraction dim
NST = S // P     # 16 s-tiles
NCH = 4          # s-chunks of 512
CH = 512
DV = 512         # v columns per core (8 heads x 64)
VROW = 528       # v tile row: [256 v | 1 one | 7 pad] x 2 heads
VOFF = 264

F32 = mybir.dt.float32
F16 = mybir.dt.float16

_CACHE = {}
LAST_RESULTS = None


def _build():
    nc = bass.Bass("TRN2")
    d_xt = nc.declare_dram_parameter("xt", [HID, S], F16, isOutput=False)
    d_wq = nc.declare_dram_parameter("wq", [HID, P], F16, isOutput=False)
    d_wk = nc.declare_dram_parameter("wk", [HID, P], F16, isOutput=False)
    d_wv = nc.declare_dram_parameter("wv", [HID, DV], F16, isOutput=False)
    d_cos = nc.declare_dram_parameter("cost", [P, S], F16, isOutput=False)
    d_sin = nc.declare_dram_parameter("sint", [P, S], F16, isOutput=False)
    d_tri = nc.declare_dram_parameter("tri", [P, P], F16, isOutput=False)
    d_out = nc.declare_dram_parameter("out", [S, DV], F16, isOutput=True)

    with tile.TileContext(nc) as tc, ExitStack() as ctx:
        pers = ctx.enter_context(tc.tile_pool(name="pers", bufs=1))
        epool = ctx.enter_context(tc.tile_pool(name="epool", bufs=64))
        outp = ctx.enter_context(tc.tile_pool(name="outp", bufs=3))
        psum = ctx.enter_context(tc.tile_pool(name="psum", bufs=8, space="PSUM"))

        def ps_tile(name):
            return psum.tile([P, CH], F32, tag="ps", bufs=8, name=name)

        # --- persistent tiles; DMA order tuned so PE never starves:
        # wq + xt0 first (first MM at ~2.8us), wk, cos/sin early (rope needs
        # them at ~31us), then the xt stream with per-k wv slices interleaved
        # (wv fully lands just before the V projection starts).
        wq_sb = pers.tile([P, NKT, P], F16, tag="wq")
        wk_sb = pers.tile([P, NKT, P], F16, tag="wk")
        wv_sb = pers.tile([P, NKT, DV], F16, tag="wv")
        cos_sb = pers.tile([P, S], F16, tag="cos")
        sin_sb = pers.tile([P, S], F16, tag="sin")
        tri_sb = pers.tile([P, P], F16, tag="tri")
        xt = [pers.tile([P, S], F16, tag=f"xt{k}", name=f"xt{k}") for k in range(NKT)]

        # Input stream split across BOTH hardware DGE queues (sync + scalar):
        # a single queue serializes on ~0.6-1.3us issue cost per DMA plus
        # semaphore recycling and stretches delivery to ~43us; two queues
        # roughly halve that. ACT is idle during phase 1 so its queue is free.
        # three streams: sync gets wq + odd xt tiles, scalar gets xt0 (in
        # parallel with wq, so the first matmul starts ~1.5us in) + even xt
        # tiles, gpsimd's SWDGE carries the whole wv stream.
        nc.sync.dma_start(out=wq_sb[:], in_=d_wq[:].rearrange("(kt p) m -> p kt m", p=P))
        nc.scalar.dma_start(out=xt[0][:], in_=d_xt[bass.ts(0, P), :])
        nc.sync.dma_start(out=xt[1][:], in_=d_xt[bass.ts(1, P), :])
        nc.scalar.dma_start(out=wk_sb[:], in_=d_wk[:].rearrange("(kt p) m -> p kt m", p=P))
        nc.sync.dma_start(out=cos_sb[:], in_=d_cos[:])
        nc.scalar.dma_start(out=xt[2][:], in_=d_xt[bass.ts(2, P), :])
        nc.sync.dma_start(out=xt[3][:], in_=d_xt[bass.ts(3, P), :])
        nc.scalar.dma_start(out=sin_sb[:], in_=d_sin[:])
        for k in range(4, NKT):
            eng = nc.scalar if k % 2 == 0 else nc.sync
            eng.dma_start(out=xt[k][:], in_=d_xt[bass.ts(k, P), :])
        nc.sync.dma_start(out=tri_sb[:], in_=d_tri[:])
        for k in range(NKT):
            nc.gpsimd.dma_start(out=wv_sb[:, k, :], in_=d_wv[bass.ts(k, P), :])

        # qt/kt are built IN PLACE: ACT/DVE evacuate the projection psums into
        # qt/kt, a DMA shuffle builds the rotate-half copy, then two in-place
        # muls + one in-place add finish RoPE with no extra tiles.
        qt = pers.tile([P, S], F16, tag="qt")
        kt = pers.tile([P, S], F16, tag="kt")
        q_rot = pers.tile([P, S], F16, tag="q_rot")
        k_rot = pers.tile([P, S], F16, tag="k_rot")
        v_sb = [pers.tile([P, VROW], F16, tag=f"v{t}", name=f"v{t}") for t in range(NST)]

        # --- warmup: keep the PE busy during the DMA lead-in so the HAM
        # clock-gate releases (1.2 -> 2.4 GHz) before the real matmuls start.
        warm_sb = pers.tile([P, CH], F16, tag="warm")
        nc.vector.memset(warm_sb[:], 0.0)
        warm_ps = ps_tile("warm")
        for i in range(6):
            nc.tensor.matmul(warm_ps[:], lhsT=warm_sb[:, 0:P], rhs=warm_sb[:],
                             start=True, stop=True, skip_group_check=True)

        # ones columns of the V tiles (denominator trick), emitted early
        for t in range(NST):
            nc.vector.memset(v_sb[t][:, 256:257], 1.0)
            nc.vector.memset(v_sb[t][:, VOFF + 256:VOFF + 257], 1.0)

        # --- phase 1: Q and K projections interleaved per xt tile (8 MMs per
        # DMA arrival keeps PE duty high while the xt stream lands).
        pq = [ps_tile(f"pq{c}") for c in range(NCH)]
        pk = [ps_tile(f"pk{c}") for c in range(NCH)]
        for k in range(NKT):
            for c in range(NCH):
                nc.tensor.matmul(pq[c][:], lhsT=wq_sb[:, k, :], rhs=xt[k][:, bass.ts(c, CH)],
                                 start=(k == 0), stop=(k == NKT - 1), skip_group_check=True)
            for c in range(NCH):
                nc.tensor.matmul(pk[c][:], lhsT=wk_sb[:, k, :], rhs=xt[k][:, bass.ts(c, CH)],
                                 start=(k == 0), stop=(k == NKT - 1), skip_group_check=True)

        # --- evacuate Q/K psums fast (frees banks for the V projection):
        # ACT takes Q, DVE takes K.
        for c in range(NCH):
            nc.scalar.copy(qt[:, bass.ts(c, CH)], pq[c][:])
        for c in range(NCH):
            nc.vector.tensor_copy(out=kt[:, bass.ts(c, CH)], in_=pk[c][:])

        # rotate-half via SBUF->SBUF DMA (off every compute engine): block b of
        # the rot tile is block b^1 of the raw tile; the sign lives in the sin
        # table. Then RoPE is 3 full-width partition-aligned in-place fp16 DVE
        # ops per tensor (walrus rejects partition-shifted tensor_tensor when
        # both inputs are SBUF, so the shift must happen in a copy).
        # shuffle DMAs go on gpsimd's SWDGE queue so they do not sit behind
        # the tail of the input stream on the hardware DGE queues
        for blk in range(4):
            lo, swap_lo = blk * 32, (blk ^ 1) * 32
            nc.gpsimd.dma_start(out=q_rot[lo:lo + 32, :], in_=qt[swap_lo:swap_lo + 32, :])
            nc.gpsimd.dma_start(out=k_rot[lo:lo + 32, :], in_=kt[swap_lo:swap_lo + 32, :])

        def rope(raw, rot):
            nc.vector.tensor_mul(out=raw[:], in0=raw[:], in1=cos_sb[:])
            nc.vector.tensor_mul(out=rot[:], in0=rot[:], in1=sin_sb[:])
            nc.vector.tensor_add(out=raw[:], in0=raw[:], in1=rot[:])

        rope(qt, q_rot)
        rope(kt, k_rot)

        # --- attention work generators (pumped between V-proj s-tiles) ---
        e_tiles = {}

        def gen_scores(c):
            cs0 = c * CH
            nk = 4 * c + 4
            tiles = [[None] * nk for _ in range(2)]
            e_tiles[c] = tiles
            for t in range(nk):
                m = t - 4 * c
                off = max(m, 0) * P
                w = CH - off
                for h in range(2):
                    ps_s = ps_tile("ps_s")
                    nc.tensor.matmul(
                        ps_s[:, 0:w],
                        lhsT=kt[h * HD:(h + 1) * HD, bass.ts(t, P)],
                        rhs=qt[h * HD:(h + 1) * HD, bass.ds(cs0 + off, w)],
                        start=True, stop=True, skip_group_check=True)
                    e = epool.tile([P, CH], F16, tag="e", name=f"e{h}_{t}")
                    nc.scalar.activation(e[:, bass.ds(off, w)], ps_s[:, 0:w],
                                         mybir.ActivationFunctionType.Exp)
                    if m >= 0:
                        nc.vector.tensor_mul(out=e[:, bass.ts(m, P)],
                                             in0=e[:, bass.ts(m, P)], in1=tri_sb[:])
                    tiles[h][t] = e
                yield 2  # 2 MMs emitted

        def gen_av(c):
            tiles = e_tiles.pop(c)
            for m in range(4):
                q_idx = 4 * c + m
                out_stage = outp.tile([P, DV], F16, tag="out_stage", name="out_stage")
                for h in range(2):
                    po = ps_tile("po")
                    for t in range(q_idx + 1):
                        nc.tensor.matmul(
                            po[:, 0:257],
                            lhsT=tiles[h][t][:, bass.ts(m, P)],
                            rhs=v_sb[t][:, h * VOFF:h * VOFF + 257],
                            start=(t == 0), stop=(t == q_idx), skip_group_check=True)
                    rec = outp.tile([P, 1], F32, tag="rec", name="rec")
                    nc.vector.reciprocal(rec[:], po[:, 256:257])
                    nc.vector.tensor_scalar_mul(
                        out_stage[:, bass.ts(h, 256)], po[:, 0:256], rec[:])
                    yield q_idx + 1
                nc.sync.dma_start(out=d_out[bass.ts(q_idx, P), :], in_=out_stage[:])

        # queue of (gate_tile, generator): emission-order gates tuned so no PE
        # instruction ever waits on rope (scores c needs rope; av c needs
        # v_sb[4c+3] and E tiles).
        # NOTE on epool sizing: 'e' allocations total 80; with 64 buffers the
        # last 16 allocations (scores(3) t>=8) wrap onto the buffers of c0/c1,
        # whose last readers (av(0)/av(1)) sit EARLIER in the PE queue -- the
        # WAR dependency points backward, so no deadlock.
        #
        # The pump round-robins across all gated-ready generators and caps
        # scores yields per call: each scores MM costs ACT a 730ns exp but PE
        # only 216ns, so long contiguous scores runs outpace ACT, stall the PE
        # on psum-bank recycling, and drop the HAM clock to 1.2 GHz.
        scores_done = {}
        work_items = [
            ["s", 0, 3, gen_scores(0), False],
            ["s", 1, 4, gen_scores(1), False],
            ["av", 0, 4, gen_av(0), False],
            ["s", 2, 5, gen_scores(2), False],
            ["av", 1, 7, gen_av(1), False],
            ["s", 3, 6, gen_scores(3), False],
            ["av", 2, 11, gen_av(2), False],
            ["av", 3, 15, gen_av(3), False],
        ]

        def pump(t_done, budget, scap):
            emitted = 0
            sc = 0
            progress = True
            while progress and emitted < budget:
                progress = False
                for item in work_items:
                    kind, cid, gate, gen, done = item
                    if done or gate > t_done:
                        continue
                    if kind == "s" and sc >= scap:
                        continue
                    if kind == "av" and not scores_done.get(cid):
                        continue
                    try:
                        emitted += next(gen)
                        if kind == "s":
                            sc += 1
                        progress = True
                    except StopIteration:
                        item[4] = True
                        if kind == "s":
                            scores_done[cid] = True
                    if emitted >= budget:
                        break
            return emitted

        # --- phase 2: V projection with attention work interleaved ---
        for t in range(NST):
            pv = ps_tile("pv")
            for k in range(NKT):
                nc.tensor.matmul(pv[:], lhsT=xt[k][:, bass.ts(t, P)], rhs=wv_sb[:, k, :],
                                 start=(k == 0), stop=(k == NKT - 1), skip_group_check=True)
            # V evac: early rounds on ACT (exp has not started; DVE must get to
            # rope immediately), later rounds on DVE (ACT saturated by exp)
            if t < 6:
                nc.scalar.copy(v_sb[t][:, 0:256], pv[:, 0:256])
                nc.scalar.copy(v_sb[t][:, VOFF:VOFF + 256], pv[:, 256:512])
            else:
                nc.vector.tensor_copy(out=v_sb[t][:, 0:256], in_=pv[:, 0:256])
                nc.vector.tensor_copy(out=v_sb[t][:, VOFF:VOFF + 256], in_=pv[:, 256:512])
            pump(t, 40, 4)
        while not all(item[4] for item in work_items):
            pump(NST, 10 ** 9, 10 ** 9)

    _legalize_waits(nc)
    _dedup_ldweights(nc)
    return nc


def _host_prep(hidden_states, position_ids, Wq, Wk, Wv):
    """Build the 8 per-core input maps."""
    hidden_states = np.asarray(hidden_states, dtype=np.float32)
    position_ids = np.asarray(position_ids)
    Wq = np.asarray(Wq, dtype=np.float32)
    Wk = np.asarray(Wk, dtype=np.float32)
    Wv = np.asarray(Wv, dtype=np.float32)

    scale = 1.0 / np.sqrt(HD)
    tri = np.triu(np.ones((P, P), dtype=np.float32)).astype(np.float16)
    inv_freq = (1.0 / (THETA ** (np.arange(0, HD, 2, dtype=np.float32) / HD))).astype(np.float32)

    in_maps = []
    for c in range(8):
        b, p = c // 4, c % 4
        xt = np.ascontiguousarray(hidden_states[b].T).astype(np.float16)
        wq = (Wq[:, p * P:(p + 1) * P] * scale).astype(np.float16)
        wk = Wk[:, p * P:(p + 1) * P].astype(np.float16)
        cols = []
        for h in (2 * p, 2 * p + 1):
            for r in range(G):
                j = r * HKV + h
                cols.append(Wv[:, j * HD:(j + 1) * HD])
        wv = np.concatenate(cols, axis=1).astype(np.float16)

        pos = position_ids[b].astype(np.float32)
        freqs = pos[:, None] * inv_freq[None, :]          # [S, 32]
        cos32 = np.cos(freqs).T.astype(np.float32)        # [32, S]
        sin32 = np.sin(freqs).T.astype(np.float32)
        cost = np.ascontiguousarray(np.tile(cos32, (4, 1))).astype(np.float16)  # [128, S]
        sint = np.ascontiguousarray(
            np.concatenate([-sin32, sin32, -sin32, sin32], axis=0)).astype(np.float16)

        in_maps.append({
            "xt": xt, "wq": wq, "wk": wk, "wv": wv,
            "cost": cost, "sint": sint, "tri": tri,
        })
    return in_maps


def kernel(hidden_states, position_ids, Wq, Wk, Wv):
    global LAST_RESULTS
    trace = bool(os.environ.get("CHEEMS_TRACE"))
    if trace:
        _install_ntff_hook()
    if "nc" not in _CACHE:
        _CACHE["nc"] = _build()
    nc = _CACHE["nc"]
    in_maps = _host_prep(hidden_states, position_ids, Wq, Wk, Wv)
    res = run_bass_kernel_spmd(nc, in_maps, core_ids=list(range(8)), trace=trace)
    LAST_RESULTS = res

    out = np.empty((B, S, HID), dtype=np.float32)
    for c in range(8):
        b, p = c // 4, c % 4
        core_out = np.asarray(res.results[c]["out"], dtype=np.float32)  # [S, 512]
        for hl, h in enumerate((2 * p, 2 * p + 1)):
            for r in range(G):
                j = r * HKV + h
                out[b, :, j * HD:(j + 1) * HD] = core_out[:, (hl * G + r) * HD:(hl * G + r + 1) * HD]
    return out.reshape(B, S, HID)


# revision 19
# speedup vs baseline: 1.0509x; 1.0509x over previous
"""Trainium2 Bass kernel for CheemsNonWoAttn (GQA attention block, no Wo).

Sharding: 8 cores = batch(2) x kv-head-pair(4). Each core handles one batch
element and 2 of the 8 kv heads (GQA: Q and K are repeated identically across
the 4 groups, so only 8 unique softmax matrices exist; V uses all 32 heads).

Per-core device program (v2, stall-free schedule):
  warmup MMs during DMA lead-in (HAM clock-gate release)
  Qt/Kt = Wq^T X^T, Wk^T X^T    (d on partitions; N=512 matmuls, K accum=16)
  psum evac split: ACT copies Q psums, DVE copies K psums -> banks free fast
  RoPE on DVE in fp16 (2x rate), 6 ops per 1024-col pair, off critical path
  V = X Wv interleaved with attention work (pump), scores gated so the
  in-order PE queue never waits on rope
  scores^T[k, q] = Kt^T Qt      (per head, K=64)
  E = exp(scores) on ACT        (no max subtraction: |scores| <~ 6, safe)
  causal mask: multiply 128x128 diagonal blocks by triangular mask (DVE)
  out = E^T V with a ones-column in V producing the softmax denominator
  out /= denom (DVE reciprocal + tensor_scalar), fp16 out, host upconverts
"""
import os
import sys
import types
from contextlib import ExitStack

for _p in ("/opt/trn_rl_repo", "/root/.axon_site/_ro/trn_rl_repo"):
    if os.path.isdir(_p) and _p not in sys.path:
        sys.path.append(_p)

import numpy as np

import concourse.bass as bass
import concourse.tile as tile
from concourse import mybir
from concourse.bass_utils import run_bass_kernel_spmd
from concourse.vector_clock import ScopedClock

# ---------------------------------------------------------------------------
# Patch 1: walrus rejects Drain instructions with >1 sync wait (CTRL ops have
# a single wait slot). Split the TileContext exit drain's waits across extra
# SP nops, one wait each.
def _patched_drain_and_barrier(self, tick_clock, wait_clock):
    nc = self.nc
    drain_bi = nc.sync.drain()
    wait_clock.add_sem_waits(drain_bi.ins, ScopedClock({None: tick_clock.global_clock}))
    inst = drain_bi.ins
    si = inst.sync_info
    if si is not None and si.on_wait is not None and len(si.on_wait) > 1:
        waits = list(si.on_wait)
        inst.sync_info = mybir.SyncInfo(
            on_wait=waits[:1],
            on_update=list(si.on_update) if si.on_update else [],
        )
        for w in waits[1:]:
            nbi = nc.sync.nop()
            nbi.ins.sync_info = mybir.SyncInfo(on_wait=[w], on_update=[])
    nc.all_engine_barrier()
    assert self.sems is not None
    popped = nc._tile_sem_poison_stack.pop()
    assert popped is self._sem_poison
    nc.clear_and_free_semaphores(list(self.sems.allocated().values()))
    if not os.environ.get("CHEEMS_TRIM_EXIT_BARRIER"):
        nc.all_engine_barrier()


tile.TileContext._drain_and_barrier = _patched_drain_and_barrier


def _legalize_waits(nc):
    """This walrus build accepts at most one sync-wait per instruction.
    Split any instruction carrying N>1 waits into N-1 preceding same-engine
    nops (engines are in-order, so semantics are preserved)."""
    uid = 0
    for f in nc.m.functions:
        for blk in f.blocks:
            insts = list(blk.instructions)
            out, changed = [], False
            for inst in insts:
                si = getattr(inst, "sync_info", None)
                if si is not None and si.on_wait is not None and len(si.on_wait) > 1:
                    waits = list(si.on_wait)
                    for w in waits[:-1]:
                        uid += 1
                        out.append(mybir.InstNoOp(
                            name=f"{inst.name}_lw{uid}",
                            engine=inst.engine,
                            sync_info=mybir.SyncInfo(on_wait=[w], on_update=[]),
                            bass_nofuse=True,
                        ))
                    inst.sync_info = mybir.SyncInfo(
                        on_wait=waits[-1:],
                        on_update=list(si.on_update) if si.on_update else [],
                    )
                    changed = True
                out.append(inst)
            if changed:
                blk.instructions = out


def _dedup_ldweights(nc):
    """Clear the self-load on matmuls whose stationary operand AP is identical
    to the immediately preceding matmul in the final PE stream (walrus runs
    with ldw-opt disabled, so it reloads weights for every matmul otherwise)."""
    if os.environ.get("CHEEMS_NO_LDW_DEDUP"):
        return
    for f in nc.m.functions:
        for blk in f.blocks:
            prev_key = None
            for inst in blk.instructions:
                if not isinstance(inst, mybir.InstMatmult):
                    if isinstance(inst, mybir.InstLdweights):
                        prev_key = None
                    continue
                key = (repr(inst.ins[1]), inst.perf_mode, inst.is_transpose,
                       repr(inst.tile_position))
                if prev_key is not None and key == prev_key:
                    inst.ldweights = False
                prev_key = key


# Patch 2 (optional, for tracing): recreate the antenv.axon_hooks shim so
# run_bass_kernel_spmd(trace=True) can capture NTFF profiles under axon.
def _install_ntff_hook():
    try:
        if "antenv.axon_hooks" in sys.modules:
            return
        import antenv
        from trn_agent_boot.trn_boot import _ntff_profile_via_ctypes

        hook = _ntff_profile_via_ctypes("/opt/axon/libaxon_pjrt.so")
        mod = types.ModuleType("antenv.axon_hooks")
        mod._hook = hook
        mod.get_axon_ntff_profile_hook = lambda: mod._hook

        def _set(h):
            mod._hook = h

        mod.set_axon_ntff_profile_hook = _set
        sys.modules["antenv.axon_hooks"] = mod
        antenv.axon_hooks = mod
    except Exception:
        pass


# ---------------------------------------------------------------------------
B, S, HID = 2, 2048, 2048
NH, G = 32, 4
HD = 64          # head dim
HKV = 8          # kv heads
THETA = 10000.0
P = 128          # partitions
NKT = HID // P   # 16 k-tiles over the contraction dim
NST = S // P     # 16 s-tiles
NCH = 4          # s-chunks of 512
CH = 512
DV = 512         # v columns per core (8 heads x 64)
VROW = 528       # v tile row: [256 v | 1 one | 7 pad] x 2 heads
VOFF = 264

F32 = mybir.dt.float32
F16 = mybir.dt.float16

_CACHE = {}
LAST_RESULTS = None


def _build():
    nc = bass.Bass("TRN2")
    d_xt = nc.declare_dram_parameter("xt", [HID, S], F16, isOutput=False)
    d_wq = nc.declare_dram_parameter("wq", [HID, P], F16, isOutput=False)
    d_wk = nc.declare_dram_parameter("wk", [HID, P], F16, isOutput=False)
    d_wv = nc.declare_dram_parameter("wv", [HID, DV], F16, isOutput=False)
    d_cos = nc.declare_dram_parameter("cost", [P, S], F16, isOutput=False)
    d_sin = nc.declare_dram_parameter("sint", [P, S], F16, isOutput=False)
    d_tri = nc.declare_dram_parameter("tri", [P, P], F16, isOutput=False)
    d_out = nc.declare_dram_parameter("out", [S, DV], F16, isOutput=True)

    with tile.TileContext(nc) as tc, ExitStack() as ctx:
        pers = ctx.enter_context(tc.tile_pool(name="pers", bufs=1))
        epool = ctx.enter_context(tc.tile_pool(name="epool", bufs=64))
        outp = ctx.enter_context(tc.tile_pool(name="outp", bufs=3))
        psum = ctx.enter_context(tc.tile_pool(name="psum", bufs=8, space="PSUM"))

        def ps_tile(name):
            return psum.tile([P, CH], F32, tag="ps", bufs=8, name=name)

        # --- persistent tiles; DMA order tuned so PE never starves:
        # wq + xt0 first (first MM at ~2.8us), wk, cos/sin early (rope needs
        # them at ~31us), then the xt stream with per-k wv slices interleaved
        # (wv fully lands just before the V projection starts).
        wq_sb = pers.tile([P, NKT, P], F16, tag="wq")
        wk_sb = pers.tile([P, NKT, P], F16, tag="wk")
        wv_sb = pers.tile([P, NKT, DV], F16, tag="wv")
        cos_sb = pers.tile([P, S], F16, tag="cos")
        sin_sb = pers.tile([P, S], F16, tag="sin")
        tri_sb = pers.tile([P, P], F16, tag="tri")
        xt = [pers.tile([P, S], F16, tag=f"xt{k}", name=f"xt{k}") for k in range(NKT)]

        # Input stream split across BOTH hardware DGE queues (sync + scalar):
        # a single queue serializes on ~0.6-1.3us issue cost per DMA plus
        # semaphore recycling and stretches delivery to ~43us; two queues
        # roughly halve that. ACT is idle during phase 1 so its queue is free.
        # two hardware-DGE streams; xt0 leads the scalar queue so it lands in
        # parallel with wq and the first matmul starts ~1.5us in. (Do NOT put
        # bulk input on gpsimd's SWDGE: it starves the HWDGE xt stream.)
        nc.sync.dma_start(out=wq_sb[:], in_=d_wq[:].rearrange("(kt p) m -> p kt m", p=P))
        nc.scalar.dma_start(out=xt[0][:], in_=d_xt[bass.ts(0, P), :])
        nc.sync.dma_start(out=xt[1][:], in_=d_xt[bass.ts(1, P), :])
        nc.scalar.dma_start(out=wk_sb[:], in_=d_wk[:].rearrange("(kt p) m -> p kt m", p=P))
        nc.sync.dma_start(out=cos_sb[:], in_=d_cos[:])
        nc.scalar.dma_start(out=xt[2][:], in_=d_xt[bass.ts(2, P), :])
        nc.sync.dma_start(out=xt[3][:], in_=d_xt[bass.ts(3, P), :])
        nc.scalar.dma_start(out=sin_sb[:], in_=d_sin[:])
        for k in range(4, NKT):
            eng = nc.scalar if k % 2 == 0 else nc.sync
            eng.dma_start(out=xt[k][:], in_=d_xt[bass.ts(k, P), :])
            eng.dma_start(out=wv_sb[:, k - 4, :], in_=d_wv[bass.ts(k - 4, P), :])
        for k in range(NKT - 4, NKT):
            eng = nc.scalar if k % 2 == 0 else nc.sync
            eng.dma_start(out=wv_sb[:, k, :], in_=d_wv[bass.ts(k, P), :])
        nc.sync.dma_start(out=tri_sb[:], in_=d_tri[:])

        # qt/kt are built IN PLACE: ACT/DVE evacuate the projection psums into
        # qt/kt, a DMA shuffle builds the rotate-half copy, then two in-place
        # muls + one in-place add finish RoPE with no extra tiles.
        qt = pers.tile([P, S], F16, tag="qt")
        kt = pers.tile([P, S], F16, tag="kt")
        q_rot = pers.tile([P, S], F16, tag="q_rot")
        k_rot = pers.tile([P, S], F16, tag="k_rot")
        v_sb = [pers.tile([P, VROW], F16, tag=f"v{t}", name=f"v{t}") for t in range(NST)]

        # --- warmup: keep the PE busy during the DMA lead-in so the HAM
        # clock-gate releases (1.2 -> 2.4 GHz) before the real matmuls start.
        warm_sb = pers.tile([P, CH], F16, tag="warm")
        nc.vector.memset(warm_sb[:], 0.0)
        warm_ps = ps_tile("warm")
        for i in range(6):
            nc.tensor.matmul(warm_ps[:], lhsT=warm_sb[:, 0:P], rhs=warm_sb[:],
                             start=True, stop=True, skip_group_check=True)

        # ones columns of the V tiles (denominator trick), emitted early
        for t in range(NST):
            nc.vector.memset(v_sb[t][:, 256:257], 1.0)
            nc.vector.memset(v_sb[t][:, VOFF + 256:VOFF + 257], 1.0)

        # --- phase 1: Q and K projections interleaved per xt tile (8 MMs per
        # DMA arrival keeps PE duty high while the xt stream lands).
        pq = [ps_tile(f"pq{c}") for c in range(NCH)]
        pk = [ps_tile(f"pk{c}") for c in range(NCH)]
        for k in range(NKT):
            for c in range(NCH):
                nc.tensor.matmul(pq[c][:], lhsT=wq_sb[:, k, :], rhs=xt[k][:, bass.ts(c, CH)],
                                 start=(k == 0), stop=(k == NKT - 1), skip_group_check=True)
            for c in range(NCH):
                nc.tensor.matmul(pk[c][:], lhsT=wk_sb[:, k, :], rhs=xt[k][:, bass.ts(c, CH)],
                                 start=(k == 0), stop=(k == NKT - 1), skip_group_check=True)

        # --- evacuate Q/K psums fast (frees banks for the V projection):
        # ACT takes Q, DVE takes K.
        for c in range(NCH):
            nc.scalar.copy(qt[:, bass.ts(c, CH)], pq[c][:])
        for c in range(NCH):
            nc.vector.tensor_copy(out=kt[:, bass.ts(c, CH)], in_=pk[c][:])

        # rotate-half via SBUF->SBUF DMA (off every compute engine): block b of
        # the rot tile is block b^1 of the raw tile; the sign lives in the sin
        # table. Then RoPE is 3 full-width partition-aligned in-place fp16 DVE
        # ops per tensor (walrus rejects partition-shifted tensor_tensor when
        # both inputs are SBUF, so the shift must happen in a copy).
        # shuffle DMAs go on gpsimd's SWDGE queue so they do not sit behind
        # the tail of the input stream on the hardware DGE queues
        for blk in range(4):
            lo, swap_lo = blk * 32, (blk ^ 1) * 32
            nc.gpsimd.dma_start(out=q_rot[lo:lo + 32, :], in_=qt[swap_lo:swap_lo + 32, :])
            nc.gpsimd.dma_start(out=k_rot[lo:lo + 32, :], in_=kt[swap_lo:swap_lo + 32, :])

        def rope(raw, rot):
            nc.vector.tensor_mul(out=raw[:], in0=raw[:], in1=cos_sb[:])
            nc.vector.tensor_mul(out=rot[:], in0=rot[:], in1=sin_sb[:])
            nc.vector.tensor_add(out=raw[:], in0=raw[:], in1=rot[:])

        rope(qt, q_rot)
        rope(kt, k_rot)

        # --- attention work generators (pumped between V-proj s-tiles) ---
        e_tiles = {}

        def gen_scores(c):
            cs0 = c * CH
            nk = 4 * c + 4
            tiles = [[None] * nk for _ in range(2)]
            e_tiles[c] = tiles
            for t in range(nk):
                m = t - 4 * c
                off = max(m, 0) * P
                w = CH - off
                for h in range(2):
                    ps_s = ps_tile("ps_s")
                    nc.tensor.matmul(
                        ps_s[:, 0:w],
                        lhsT=kt[h * HD:(h + 1) * HD, bass.ts(t, P)],
                        rhs=qt[h * HD:(h + 1) * HD, bass.ds(cs0 + off, w)],
                        start=True, stop=True, skip_group_check=True)
                    e = epool.tile([P, CH], F16, tag="e", name=f"e{h}_{t}")
                    nc.scalar.activation(e[:, bass.ds(off, w)], ps_s[:, 0:w],
                                         mybir.ActivationFunctionType.Exp)
                    if m >= 0:
                        nc.vector.tensor_mul(out=e[:, bass.ts(m, P)],
                                             in0=e[:, bass.ts(m, P)], in1=tri_sb[:])
                    tiles[h][t] = e
                yield 2  # 2 MMs emitted

        def gen_av(c):
            tiles = e_tiles.pop(c)
            for m in range(4):
                q_idx = 4 * c + m
                out_stage = outp.tile([P, DV], F16, tag="out_stage", name="out_stage")
                for h in range(2):
                    po = ps_tile("po")
                    for t in range(q_idx + 1):
                        nc.tensor.matmul(
                            po[:, 0:257],
                            lhsT=tiles[h][t][:, bass.ts(m, P)],
                            rhs=v_sb[t][:, h * VOFF:h * VOFF + 257],
                            start=(t == 0), stop=(t == q_idx), skip_group_check=True)
                    rec = outp.tile([P, 1], F32, tag="rec", name="rec")
                    nc.vector.reciprocal(rec[:], po[:, 256:257])
                    nc.vector.tensor_scalar_mul(
                        out_stage[:, bass.ts(h, 256)], po[:, 0:256], rec[:])
                    yield q_idx + 1
                nc.sync.dma_start(out=d_out[bass.ts(q_idx, P), :], in_=out_stage[:])

        # queue of (gate_tile, generator): emission-order gates tuned so no PE
        # instruction ever waits on rope (scores c needs rope; av c needs
        # v_sb[4c+3] and E tiles).
        # NOTE on epool sizing: 'e' allocations total 80; with 64 buffers the
        # last 16 allocations (scores(3) t>=8) wrap onto the buffers of c0/c1,
        # whose last readers (av(0)/av(1)) sit EARLIER in the PE queue -- the
        # WAR dependency points backward, so no deadlock.
        #
        # The pump round-robins across all gated-ready generators and caps
        # scores yields per call: each scores MM costs ACT a 730ns exp but PE
        # only 216ns, so long contiguous scores runs outpace ACT, stall the PE
        # on psum-bank recycling, and drop the HAM clock to 1.2 GHz.
        scores_done = {}
        work_items = [
            ["s", 0, 3, gen_scores(0), False],
            ["s", 1, 4, gen_scores(1), False],
            ["av", 0, 4, gen_av(0), False],
            ["s", 2, 5, gen_scores(2), False],
            ["av", 1, 7, gen_av(1), False],
            ["s", 3, 6, gen_scores(3), False],
            ["av", 2, 11, gen_av(2), False],
            ["av", 3, 15, gen_av(3), False],
        ]

        def pump(t_done, budget, scap):
            emitted = 0
            sc = 0
            progress = True
            while progress and emitted < budget:
                progress = False
                for item in work_items:
                    kind, cid, gate, gen, done = item
                    if done or gate > t_done:
                        continue
                    if kind == "s" and sc >= scap:
                        continue
                    if kind == "av" and not scores_done.get(cid):
                        continue
                    try:
                        emitted += next(gen)
                        if kind == "s":
                            sc += 1
                        progress = True
                    except StopIteration:
                        item[4] = True
                        if kind == "s":
                            scores_done[cid] = True
                    if emitted >= budget:
                        break
            return emitted

        # --- phase 2: V projection with attention work interleaved ---
        for t in range(NST):
            pv = ps_tile("pv")
            for k in range(NKT):
                nc.tensor.matmul(pv[:], lhsT=xt[k][:, bass.ts(t, P)], rhs=wv_sb[:, k, :],
                                 start=(k == 0), stop=(k == NKT - 1), skip_group_check=True)
            # V evac: early rounds on ACT (exp has not started; DVE must get to
            # rope immediately), later rounds on DVE (ACT saturated by exp)
            if t < 6:
                nc.scalar.copy(v_sb[t][:, 0:256], pv[:, 0:256])
                nc.scalar.copy(v_sb[t][:, VOFF:VOFF + 256], pv[:, 256:512])
            else:
                nc.vector.tensor_copy(out=v_sb[t][:, 0:256], in_=pv[:, 0:256])
                nc.vector.tensor_copy(out=v_sb[t][:, VOFF:VOFF + 256], in_=pv[:, 256:512])
            pump(t, 40, 4)
        while not all(item[4] for item in work_items):
            pump(NST, 10 ** 9, 10 ** 9)

    _legalize_waits(nc)
    _dedup_ldweights(nc)
    return nc


def _host_prep(hidden_states, position_ids, Wq, Wk, Wv):
    """Build the 8 per-core input maps."""
    hidden_states = np.asarray(hidden_states, dtype=np.float32)
    position_ids = np.asarray(position_ids)
    Wq = np.asarray(Wq, dtype=np.float32)
    Wk = np.asarray(Wk, dtype=np.float32)
    Wv = np.asarray(Wv, dtype=np.float32)

    scale = 1.0 / np.sqrt(HD)
    tri = np.triu(np.ones((P, P), dtype=np.float32)).astype(np.float16)
    inv_freq = (1.0 / (THETA ** (np.arange(0, HD, 2, dtype=np.float32) / HD))).astype(np.float32)

    in_maps = []
    for c in range(8):
        b, p = c // 4, c % 4
        xt = np.ascontiguousarray(hidden_states[b].T).astype(np.float16)
        wq = (Wq[:, p * P:(p + 1) * P] * scale).astype(np.float16)
        wk = Wk[:, p * P:(p + 1) * P].astype(np.float16)
        cols = []
        for h in (2 * p, 2 * p + 1):
            for r in range(G):
                j = r * HKV + h
                cols.append(Wv[:, j * HD:(j + 1) * HD])
        wv = np.concatenate(cols, axis=1).astype(np.float16)

        pos = position_ids[b].astype(np.float32)
        freqs = pos[:, None] * inv_freq[None, :]          # [S, 32]
        cos32 = np.cos(freqs).T.astype(np.float32)        # [32, S]
        sin32 = np.sin(freqs).T.astype(np.float32)
        cost = np.ascontiguousarray(np.tile(cos32, (4, 1))).astype(np.float16)  # [128, S]
        sint = np.ascontiguousarray(
            np.concatenate([-sin32, sin32, -sin32, sin32], axis=0)).astype(np.float16)

        in_maps.append({
            "xt": xt, "wq": wq, "wk": wk, "wv": wv,
            "cost": cost, "sint": sint, "tri": tri,
        })
    return in_maps


def kernel(hidden_states, position_ids, Wq, Wk, Wv):
    global LAST_RESULTS
    trace = bool(os.environ.get("CHEEMS_TRACE"))
    if trace:
        _install_ntff_hook()
    if "nc" not in _CACHE:
        _CACHE["nc"] = _build()
    nc = _CACHE["nc"]
    in_maps = _host_prep(hidden_states, position_ids, Wq, Wk, Wv)
    res = run_bass_kernel_spmd(nc, in_maps, core_ids=list(range(8)), trace=trace)
    LAST_RESULTS = res

    out = np.empty((B, S, HID), dtype=np.float32)
    for c in range(8):
        b, p = c // 4, c % 4
        core_out = np.asarray(res.results[c]["out"], dtype=np.float32)  # [S, 512]
        for hl, h in enumerate((2 * p, 2 * p + 1)):
            for r in range(G):
                j = r * HKV + h
                out[b, :, j * HD:(j + 1) * HD] = core_out[:, (hl * G + r) * HD:(hl * G + r + 1) * HD]
    return out.reshape(B, S, HID)


# revision 20
# speedup vs baseline: 1.0898x; 1.0370x over previous
"""Trainium2 Bass kernel for CheemsNonWoAttn (GQA attention block, no Wo).

Sharding: 8 cores = batch(2) x kv-head-pair(4). Each core handles one batch
element and 2 of the 8 kv heads (GQA: Q and K are repeated identically across
the 4 groups, so only 8 unique softmax matrices exist; V uses all 32 heads).

Per-core device program (v2, stall-free schedule):
  warmup MMs during DMA lead-in (HAM clock-gate release)
  Qt/Kt = Wq^T X^T, Wk^T X^T    (d on partitions; N=512 matmuls, K accum=16)
  psum evac split: ACT copies Q psums, DVE copies K psums -> banks free fast
  RoPE on DVE in fp16 (2x rate), 6 ops per 1024-col pair, off critical path
  V = X Wv interleaved with attention work (pump), scores gated so the
  in-order PE queue never waits on rope
  scores^T[k, q] = Kt^T Qt      (per head, K=64)
  E = exp(scores) on ACT        (no max subtraction: |scores| <~ 6, safe)
  causal mask: multiply 128x128 diagonal blocks by triangular mask (DVE)
  out = E^T V with a ones-column in V producing the softmax denominator
  out /= denom (DVE reciprocal + tensor_scalar), fp16 out, host upconverts
"""
import os
import sys
import types
from contextlib import ExitStack

for _p in ("/opt/trn_rl_repo", "/root/.axon_site/_ro/trn_rl_repo"):
    if os.path.isdir(_p) and _p not in sys.path:
        sys.path.append(_p)

import numpy as np

import concourse.bass as bass
import concourse.tile as tile
from concourse import mybir
from concourse.bass_utils import run_bass_kernel_spmd
from concourse.vector_clock import ScopedClock

# ---------------------------------------------------------------------------
# Patch 1: walrus rejects Drain instructions with >1 sync wait (CTRL ops have
# a single wait slot). Split the TileContext exit drain's waits across extra
# SP nops, one wait each.
def _patched_drain_and_barrier(self, tick_clock, wait_clock):
    nc = self.nc
    drain_bi = nc.sync.drain()
    wait_clock.add_sem_waits(drain_bi.ins, ScopedClock({None: tick_clock.global_clock}))
    inst = drain_bi.ins
    si = inst.sync_info
    if si is not None and si.on_wait is not None and len(si.on_wait) > 1:
        waits = list(si.on_wait)
        inst.sync_info = mybir.SyncInfo(
            on_wait=waits[:1],
            on_update=list(si.on_update) if si.on_update else [],
        )
        for w in waits[1:]:
            nbi = nc.sync.nop()
            nbi.ins.sync_info = mybir.SyncInfo(on_wait=[w], on_update=[])
    nc.all_engine_barrier()
    assert self.sems is not None
    popped = nc._tile_sem_poison_stack.pop()
    assert popped is self._sem_poison
    nc.clear_and_free_semaphores(list(self.sems.allocated().values()))
    if not os.environ.get("CHEEMS_TRIM_EXIT_BARRIER"):
        nc.all_engine_barrier()


tile.TileContext._drain_and_barrier = _patched_drain_and_barrier


def _legalize_waits(nc):
    """This walrus build accepts at most one sync-wait per instruction.
    Split any instruction carrying N>1 waits into N-1 preceding same-engine
    nops (engines are in-order, so semantics are preserved)."""
    uid = 0
    for f in nc.m.functions:
        for blk in f.blocks:
            insts = list(blk.instructions)
            out, changed = [], False
            for inst in insts:
                si = getattr(inst, "sync_info", None)
                if si is not None and si.on_wait is not None and len(si.on_wait) > 1:
                    waits = list(si.on_wait)
                    for w in waits[:-1]:
                        uid += 1
                        out.append(mybir.InstNoOp(
                            name=f"{inst.name}_lw{uid}",
                            engine=inst.engine,
                            sync_info=mybir.SyncInfo(on_wait=[w], on_update=[]),
                            bass_nofuse=True,
                        ))
                    inst.sync_info = mybir.SyncInfo(
                        on_wait=waits[-1:],
                        on_update=list(si.on_update) if si.on_update else [],
                    )
                    changed = True
                out.append(inst)
            if changed:
                blk.instructions = out


def _dedup_ldweights(nc):
    """Clear the self-load on matmuls whose stationary operand AP is identical
    to the immediately preceding matmul in the final PE stream (walrus runs
    with ldw-opt disabled, so it reloads weights for every matmul otherwise)."""
    if os.environ.get("CHEEMS_NO_LDW_DEDUP"):
        return
    for f in nc.m.functions:
        for blk in f.blocks:
            prev_key = None
            for inst in blk.instructions:
                if not isinstance(inst, mybir.InstMatmult):
                    if isinstance(inst, mybir.InstLdweights):
                        prev_key = None
                    continue
                key = (repr(inst.ins[1]), inst.perf_mode, inst.is_transpose,
                       repr(inst.tile_position))
                if prev_key is not None and key == prev_key:
                    inst.ldweights = False
                prev_key = key


# Patch 2 (optional, for tracing): recreate the antenv.axon_hooks shim so
# run_bass_kernel_spmd(trace=True) can capture NTFF profiles under axon.
def _install_ntff_hook():
    try:
        if "antenv.axon_hooks" in sys.modules:
            return
        import antenv
        from trn_agent_boot.trn_boot import _ntff_profile_via_ctypes

        hook = _ntff_profile_via_ctypes("/opt/axon/libaxon_pjrt.so")
        mod = types.ModuleType("antenv.axon_hooks")
        mod._hook = hook
        mod.get_axon_ntff_profile_hook = lambda: mod._hook

        def _set(h):
            mod._hook = h

        mod.set_axon_ntff_profile_hook = _set
        sys.modules["antenv.axon_hooks"] = mod
        antenv.axon_hooks = mod
    except Exception:
        pass


# ---------------------------------------------------------------------------
B, S, HID = 2, 2048, 2048
NH, G = 32, 4
HD = 64          # head dim
HKV = 8          # kv heads
THETA = 10000.0
P = 128          # partitions
NKT = HID // P   # 16 k-tiles over the contraction dim
NST = S // P     # 16 s-tiles
NCH = 4          # s-chunks of 512
CH = 512
DV = 512         # v columns per core (8 heads x 64)
VROW = 528       # v tile row: [256 v | 1 one | 7 pad] x 2 heads
VOFF = 264

F32 = mybir.dt.float32
F16 = mybir.dt.float16

_CACHE = {}
LAST_RESULTS = None


def _build():
    nc = bass.Bass("TRN2")
    d_xt = nc.declare_dram_parameter("xt", [HID, S], F16, isOutput=False)
    d_wq = nc.declare_dram_parameter("wq", [HID, P], F16, isOutput=False)
    d_wk = nc.declare_dram_parameter("wk", [HID, P], F16, isOutput=False)
    d_wv = nc.declare_dram_parameter("wv", [HID, DV], F16, isOutput=False)
    d_cos = nc.declare_dram_parameter("cost", [P, S], F16, isOutput=False)
    d_sin = nc.declare_dram_parameter("sint", [P, S], F16, isOutput=False)
    d_tri = nc.declare_dram_parameter("tri", [P, P], F16, isOutput=False)
    d_out = nc.declare_dram_parameter("out", [S, DV], F16, isOutput=True)

    with tile.TileContext(nc) as tc, ExitStack() as ctx:
        pers = ctx.enter_context(tc.tile_pool(name="pers", bufs=1))
        epool = ctx.enter_context(tc.tile_pool(name="epool", bufs=64))
        outp = ctx.enter_context(tc.tile_pool(name="outp", bufs=3))
        psum = ctx.enter_context(tc.tile_pool(name="psum", bufs=8, space="PSUM"))

        def ps_tile(name):
            return psum.tile([P, CH], F32, tag="ps", bufs=8, name=name)

        # --- persistent tiles; DMA order tuned so PE never starves:
        # wq + xt0 first (first MM at ~2.8us), wk, cos/sin early (rope needs
        # them at ~31us), then the xt stream with per-k wv slices interleaved
        # (wv fully lands just before the V projection starts).
        wq_sb = pers.tile([P, NKT, P], F16, tag="wq")
        wk_sb = pers.tile([P, NKT, P], F16, tag="wk")
        wv_sb = pers.tile([P, NKT, DV], F16, tag="wv")
        cos_sb = pers.tile([P, S], F16, tag="cos")
        sin_sb = pers.tile([P, S], F16, tag="sin")
        tri_sb = pers.tile([P, P], F16, tag="tri")
        xt = [pers.tile([P, S], F16, tag=f"xt{k}", name=f"xt{k}") for k in range(NKT)]

        # Input stream split across BOTH hardware DGE queues (sync + scalar):
        # a single queue serializes on ~0.6-1.3us issue cost per DMA plus
        # semaphore recycling and stretches delivery to ~43us; two queues
        # roughly halve that. ACT is idle during phase 1 so its queue is free.
        # two hardware-DGE streams (sync + scalar), interleaved xt/wv. Do NOT
        # put bulk input on gpsimd's SWDGE: it starves the HWDGE xt stream.
        nc.sync.dma_start(out=wq_sb[:], in_=d_wq[:].rearrange("(kt p) m -> p kt m", p=P))
        nc.scalar.dma_start(out=wk_sb[:], in_=d_wk[:].rearrange("(kt p) m -> p kt m", p=P))
        nc.sync.dma_start(out=xt[0][:], in_=d_xt[bass.ts(0, P), :])
        nc.scalar.dma_start(out=xt[1][:], in_=d_xt[bass.ts(1, P), :])
        nc.sync.dma_start(out=cos_sb[:], in_=d_cos[:])
        nc.scalar.dma_start(out=sin_sb[:], in_=d_sin[:])
        for k in range(2, NKT):
            eng = nc.sync if k % 2 == 0 else nc.scalar
            eng.dma_start(out=xt[k][:], in_=d_xt[bass.ts(k, P), :])
            eng.dma_start(out=wv_sb[:, k - 2, :], in_=d_wv[bass.ts(k - 2, P), :])
        nc.sync.dma_start(out=wv_sb[:, NKT - 2, :], in_=d_wv[bass.ts(NKT - 2, P), :])
        nc.scalar.dma_start(out=wv_sb[:, NKT - 1, :], in_=d_wv[bass.ts(NKT - 1, P), :])
        nc.sync.dma_start(out=tri_sb[:], in_=d_tri[:])

        # qt/kt are built IN PLACE: ACT/DVE evacuate the projection psums into
        # qt/kt, a DMA shuffle builds the rotate-half copy, then two in-place
        # muls + one in-place add finish RoPE with no extra tiles.
        qt = pers.tile([P, S], F16, tag="qt")
        kt = pers.tile([P, S], F16, tag="kt")
        q_rot = pers.tile([P, S], F16, tag="q_rot")
        k_rot = pers.tile([P, S], F16, tag="k_rot")
        v_sb = [pers.tile([P, VROW], F16, tag=f"v{t}", name=f"v{t}") for t in range(NST)]

        # --- warmup: keep the PE busy during the DMA lead-in so the HAM
        # clock-gate releases (1.2 -> 2.4 GHz) before the real matmuls start.
        warm_sb = pers.tile([P, CH], F16, tag="warm")
        nc.vector.memset(warm_sb[:], 0.0)
        warm_ps = ps_tile("warm")
        for i in range(6):
            nc.tensor.matmul(warm_ps[:], lhsT=warm_sb[:, 0:P], rhs=warm_sb[:],
                             start=True, stop=True, skip_group_check=True)

        # ones columns of the V tiles (denominator trick), emitted early
        for t in range(NST):
            nc.vector.memset(v_sb[t][:, 256:257], 1.0)
            nc.vector.memset(v_sb[t][:, VOFF + 256:VOFF + 257], 1.0)

        # --- phase 1: Q and K projections interleaved per xt tile (8 MMs per
        # DMA arrival keeps PE duty high while the xt stream lands).
        pq = [ps_tile(f"pq{c}") for c in range(NCH)]
        pk = [ps_tile(f"pk{c}") for c in range(NCH)]
        for k in range(NKT):
            for c in range(NCH):
                nc.tensor.matmul(pq[c][:], lhsT=wq_sb[:, k, :], rhs=xt[k][:, bass.ts(c, CH)],
                                 start=(k == 0), stop=(k == NKT - 1), skip_group_check=True)
            for c in range(NCH):
                nc.tensor.matmul(pk[c][:], lhsT=wk_sb[:, k, :], rhs=xt[k][:, bass.ts(c, CH)],
                                 start=(k == 0), stop=(k == NKT - 1), skip_group_check=True)

        # --- evacuate Q/K psums fast (frees banks for the V projection):
        # ACT takes Q, DVE takes K.
        for c in range(NCH):
            nc.scalar.copy(qt[:, bass.ts(c, CH)], pq[c][:])
        for c in range(NCH):
            nc.vector.tensor_copy(out=kt[:, bass.ts(c, CH)], in_=pk[c][:])

        # rotate-half via SBUF->SBUF DMA (off every compute engine): block b of
        # the rot tile is block b^1 of the raw tile; the sign lives in the sin
        # table. Then RoPE is 3 full-width partition-aligned in-place fp16 DVE
        # ops per tensor (walrus rejects partition-shifted tensor_tensor when
        # both inputs are SBUF, so the shift must happen in a copy).
        # shuffle DMAs go on gpsimd's SWDGE queue so they do not sit behind
        # the tail of the input stream on the hardware DGE queues
        for blk in range(4):
            lo, swap_lo = blk * 32, (blk ^ 1) * 32
            nc.gpsimd.dma_start(out=q_rot[lo:lo + 32, :], in_=qt[swap_lo:swap_lo + 32, :])
            nc.gpsimd.dma_start(out=k_rot[lo:lo + 32, :], in_=kt[swap_lo:swap_lo + 32, :])

        def rope(raw, rot):
            nc.vector.tensor_mul(out=raw[:], in0=raw[:], in1=cos_sb[:])
            nc.vector.tensor_mul(out=rot[:], in0=rot[:], in1=sin_sb[:])
            nc.vector.tensor_add(out=raw[:], in0=raw[:], in1=rot[:])

        rope(qt, q_rot)
        rope(kt, k_rot)

        # --- attention work generators (pumped between V-proj s-tiles) ---
        e_tiles = {}

        def gen_scores(c):
            cs0 = c * CH
            nk = 4 * c + 4
            tiles = [[None] * nk for _ in range(2)]
            e_tiles[c] = tiles
            for t in range(nk):
                m = t - 4 * c
                off = max(m, 0) * P
                w = CH - off
                for h in range(2):
                    ps_s = ps_tile("ps_s")
                    nc.tensor.matmul(
                        ps_s[:, 0:w],
                        lhsT=kt[h * HD:(h + 1) * HD, bass.ts(t, P)],
                        rhs=qt[h * HD:(h + 1) * HD, bass.ds(cs0 + off, w)],
                        start=True, stop=True, skip_group_check=True)
                    e = epool.tile([P, CH], F16, tag="e", name=f"e{h}_{t}")
                    nc.scalar.activation(e[:, bass.ds(off, w)], ps_s[:, 0:w],
                                         mybir.ActivationFunctionType.Exp)
                    if m >= 0:
                        nc.vector.tensor_mul(out=e[:, bass.ts(m, P)],
                                             in0=e[:, bass.ts(m, P)], in1=tri_sb[:])
                    tiles[h][t] = e
                yield 2  # 2 MMs emitted

        def gen_av(c):
            tiles = e_tiles.pop(c)
            for m in range(4):
                q_idx = 4 * c + m
                out_stage = outp.tile([P, DV], F16, tag="out_stage", name="out_stage")
                for h in range(2):
                    po = ps_tile("po")
                    for t in range(q_idx + 1):
                        nc.tensor.matmul(
                            po[:, 0:257],
                            lhsT=tiles[h][t][:, bass.ts(m, P)],
                            rhs=v_sb[t][:, h * VOFF:h * VOFF + 257],
                            start=(t == 0), stop=(t == q_idx), skip_group_check=True)
                    rec = outp.tile([P, 1], F32, tag="rec", name="rec")
                    nc.vector.reciprocal(rec[:], po[:, 256:257])
                    nc.vector.tensor_scalar_mul(
                        out_stage[:, bass.ts(h, 256)], po[:, 0:256], rec[:])
                    yield q_idx + 1
                nc.sync.dma_start(out=d_out[bass.ts(q_idx, P), :], in_=out_stage[:])

        # queue of (gate_tile, generator): emission-order gates tuned so no PE
        # instruction ever waits on rope (scores c needs rope; av c needs
        # v_sb[4c+3] and E tiles).
        # NOTE on epool sizing: 'e' allocations total 80; with 64 buffers the
        # last 16 allocations (scores(3) t>=8) wrap onto the buffers of c0/c1,
        # whose last readers (av(0)/av(1)) sit EARLIER in the PE queue -- the
        # WAR dependency points backward, so no deadlock.
        #
        # The pump round-robins across all gated-ready generators and caps
        # scores yields per call: each scores MM costs ACT a 730ns exp but PE
        # only 216ns, so long contiguous scores runs outpace ACT, stall the PE
        # on psum-bank recycling, and drop the HAM clock to 1.2 GHz.
        scores_done = {}
        work_items = [
            ["s", 0, 3, gen_scores(0), False],
            ["s", 1, 4, gen_scores(1), False],
            ["av", 0, 4, gen_av(0), False],
            ["s", 2, 5, gen_scores(2), False],
            ["av", 1, 7, gen_av(1), False],
            ["s", 3, 6, gen_scores(3), False],
            ["av", 2, 11, gen_av(2), False],
            ["av", 3, 15, gen_av(3), False],
        ]

        def pump(t_done, budget, scap):
            emitted = 0
            sc = 0
            progress = True
            while progress and emitted < budget:
                progress = False
                for item in work_items:
                    kind, cid, gate, gen, done = item
                    if done or gate > t_done:
                        continue
                    if kind == "s" and sc >= scap:
                        continue
                    if kind == "av" and not scores_done.get(cid):
                        continue
                    try:
                        emitted += next(gen)
                        if kind == "s":
                            sc += 1
                        progress = True
                    except StopIteration:
                        item[4] = True
                        if kind == "s":
                            scores_done[cid] = True
                    if emitted >= budget:
                        break
            return emitted

        # --- phase 2: V projection with attention work interleaved ---
        for t in range(NST):
            pv = ps_tile("pv")
            for k in range(NKT):
                nc.tensor.matmul(pv[:], lhsT=xt[k][:, bass.ts(t, P)], rhs=wv_sb[:, k, :],
                                 start=(k == 0), stop=(k == NKT - 1), skip_group_check=True)
            # V evac: early rounds on ACT (exp has not started; DVE must get to
            # rope immediately), later rounds on DVE (ACT saturated by exp)
            if t < 6:
                nc.scalar.copy(v_sb[t][:, 0:256], pv[:, 0:256])
                nc.scalar.copy(v_sb[t][:, VOFF:VOFF + 256], pv[:, 256:512])
            else:
                nc.vector.tensor_copy(out=v_sb[t][:, 0:256], in_=pv[:, 0:256])
                nc.vector.tensor_copy(out=v_sb[t][:, VOFF:VOFF + 256], in_=pv[:, 256:512])
            pump(t, 40, 4)
        while not all(item[4] for item in work_items):
            pump(NST, 10 ** 9, 10 ** 9)

    _legalize_waits(nc)
    _dedup_ldweights(nc)
    return nc


def _host_prep(hidden_states, position_ids, Wq, Wk, Wv):
    """Build the 8 per-core input maps."""
    hidden_states = np.asarray(hidden_states, dtype=np.float32)
    position_ids = np.asarray(position_ids)
    Wq = np.asarray(Wq, dtype=np.float32)
    Wk = np.asarray(Wk, dtype=np.float32)
    Wv = np.asarray(Wv, dtype=np.float32)

    scale = 1.0 / np.sqrt(HD)
    tri = np.triu(np.ones((P, P), dtype=np.float32)).astype(np.float16)
    inv_freq = (1.0 / (THETA ** (np.arange(0, HD, 2, dtype=np.float32) / HD))).astype(np.float32)

    in_maps = []
    for c in range(8):
        b, p = c // 4, c % 4
        xt = np.ascontiguousarray(hidden_states[b].T).astype(np.float16)
        wq = (Wq[:, p * P:(p + 1) * P] * scale).astype(np.float16)
        wk = Wk[:, p * P:(p + 1) * P].astype(np.float16)
        cols = []
        for h in (2 * p, 2 * p + 1):
            for r in range(G):
                j = r * HKV + h
                cols.append(Wv[:, j * HD:(j + 1) * HD])
        wv = np.concatenate(cols, axis=1).astype(np.float16)

        pos = position_ids[b].astype(np.float32)
        freqs = pos[:, None] * inv_freq[None, :]          # [S, 32]
        cos32 = np.cos(freqs).T.astype(np.float32)        # [32, S]
        sin32 = np.sin(freqs).T.astype(np.float32)
        cost = np.ascontiguousarray(np.tile(cos32, (4, 1))).astype(np.float16)  # [128, S]
        sint = np.ascontiguousarray(
            np.concatenate([-sin32, sin32, -sin32, sin32], axis=0)).astype(np.float16)

        in_maps.append({
            "xt": xt, "wq": wq, "wk": wk, "wv": wv,
            "cost": cost, "sint": sint, "tri": tri,
        })
    return in_maps


def kernel(hidden_states, position_ids, Wq, Wk, Wv):
    global LAST_RESULTS
    trace = bool(os.environ.get("CHEEMS_TRACE"))
    if trace:
        _install_ntff_hook()
    if "nc" not in _CACHE:
        _CACHE["nc"] = _build()
    nc = _CACHE["nc"]
    in_maps = _host_prep(hidden_states, position_ids, Wq, Wk, Wv)
    res = run_bass_kernel_spmd(nc, in_maps, core_ids=list(range(8)), trace=trace)
    LAST_RESULTS = res

    out = np.empty((B, S, HID), dtype=np.float32)
    for c in range(8):
        b, p = c // 4, c % 4
        core_out = np.asarray(res.results[c]["out"], dtype=np.float32)  # [S, 512]
        for hl, h in enumerate((2 * p, 2 * p + 1)):
            for r in range(G):
                j = r * HKV + h
                out[b, :, j * HD:(j + 1) * HD] = core_out[:, (hl * G + r) * HD:(hl * G + r + 1) * HD]
    return out.reshape(B, S, HID)


# revision 22
# speedup vs baseline: 1.1213x; 1.0289x over previous
"""Trainium2 Bass kernel for CheemsNonWoAttn (GQA attention block, no Wo).

Sharding: 8 cores = batch(2) x kv-head-pair(4). Each core handles one batch
element and 2 of the 8 kv heads (GQA: Q and K are repeated identically across
the 4 groups, so only 8 unique softmax matrices exist; V uses all 32 heads).

Per-core device program (v2, stall-free schedule):
  warmup MMs during DMA lead-in (HAM clock-gate release)
  Qt/Kt = Wq^T X^T, Wk^T X^T    (d on partitions; N=512 matmuls, K accum=16)
  psum evac split: ACT copies Q psums, DVE copies K psums -> banks free fast
  RoPE on DVE in fp16 (2x rate), 6 ops per 1024-col pair, off critical path
  V = X Wv interleaved with attention work (pump), scores gated so the
  in-order PE queue never waits on rope
  scores^T[k, q] = Kt^T Qt      (per head, K=64)
  E = exp(scores) on ACT        (no max subtraction: |scores| <~ 6, safe)
  causal mask: multiply 128x128 diagonal blocks by triangular mask (DVE)
  out = E^T V with a ones-column in V producing the softmax denominator
  out /= denom (DVE reciprocal + tensor_scalar), fp16 out, host upconverts
"""
import os
import sys
import types
from contextlib import ExitStack

for _p in ("/opt/trn_rl_repo", "/root/.axon_site/_ro/trn_rl_repo"):
    if os.path.isdir(_p) and _p not in sys.path:
        sys.path.append(_p)

import numpy as np

import concourse.bass as bass
import concourse.tile as tile
from concourse import mybir
from concourse.bass_utils import run_bass_kernel_spmd
from concourse.vector_clock import ScopedClock

# ---------------------------------------------------------------------------
# Patch 1: walrus rejects Drain instructions with >1 sync wait (CTRL ops have
# a single wait slot). Split the TileContext exit drain's waits across extra
# SP nops, one wait each.
def _patched_drain_and_barrier(self, tick_clock, wait_clock):
    nc = self.nc
    drain_bi = nc.sync.drain()
    wait_clock.add_sem_waits(drain_bi.ins, ScopedClock({None: tick_clock.global_clock}))
    inst = drain_bi.ins
    si = inst.sync_info
    if si is not None and si.on_wait is not None and len(si.on_wait) > 1:
        waits = list(si.on_wait)
        inst.sync_info = mybir.SyncInfo(
            on_wait=waits[:1],
            on_update=list(si.on_update) if si.on_update else [],
        )
        for w in waits[1:]:
            nbi = nc.sync.nop()
            nbi.ins.sync_info = mybir.SyncInfo(on_wait=[w], on_update=[])
    nc.all_engine_barrier()
    assert self.sems is not None
    popped = nc._tile_sem_poison_stack.pop()
    assert popped is self._sem_poison
    nc.clear_and_free_semaphores(list(self.sems.allocated().values()))
    if not os.environ.get("CHEEMS_TRIM_EXIT_BARRIER"):
        nc.all_engine_barrier()


tile.TileContext._drain_and_barrier = _patched_drain_and_barrier


def _legalize_waits(nc):
    """This walrus build accepts at most one sync-wait per instruction.
    Split any instruction carrying N>1 waits into N-1 preceding same-engine
    nops (engines are in-order, so semantics are preserved)."""
    uid = 0
    for f in nc.m.functions:
        for blk in f.blocks:
            insts = list(blk.instructions)
            out, changed = [], False
            for inst in insts:
                si = getattr(inst, "sync_info", None)
                if si is not None and si.on_wait is not None and len(si.on_wait) > 1:
                    waits = list(si.on_wait)
                    for w in waits[:-1]:
                        uid += 1
                        out.append(mybir.InstNoOp(
                            name=f"{inst.name}_lw{uid}",
                            engine=inst.engine,
                            sync_info=mybir.SyncInfo(on_wait=[w], on_update=[]),
                            bass_nofuse=True,
                        ))
                    inst.sync_info = mybir.SyncInfo(
                        on_wait=waits[-1:],
                        on_update=list(si.on_update) if si.on_update else [],
                    )
                    changed = True
                out.append(inst)
            if changed:
                blk.instructions = out


def _dedup_ldweights(nc):
    """Clear the self-load on matmuls whose stationary operand AP is identical
    to the immediately preceding matmul in the final PE stream (walrus runs
    with ldw-opt disabled, so it reloads weights for every matmul otherwise)."""
    if os.environ.get("CHEEMS_NO_LDW_DEDUP"):
        return
    for f in nc.m.functions:
        for blk in f.blocks:
            prev_key = None
            for inst in blk.instructions:
                if not isinstance(inst, mybir.InstMatmult):
                    if isinstance(inst, mybir.InstLdweights):
                        prev_key = None
                    continue
                key = (repr(inst.ins[1]), inst.perf_mode, inst.is_transpose,
                       repr(inst.tile_position))
                if prev_key is not None and key == prev_key:
                    inst.ldweights = False
                prev_key = key


# Patch 2 (optional, for tracing): recreate the antenv.axon_hooks shim so
# run_bass_kernel_spmd(trace=True) can capture NTFF profiles under axon.
def _install_ntff_hook():
    try:
        if "antenv.axon_hooks" in sys.modules:
            return
        import antenv
        from trn_agent_boot.trn_boot import _ntff_profile_via_ctypes

        hook = _ntff_profile_via_ctypes("/opt/axon/libaxon_pjrt.so")
        mod = types.ModuleType("antenv.axon_hooks")
        mod._hook = hook
        mod.get_axon_ntff_profile_hook = lambda: mod._hook

        def _set(h):
            mod._hook = h

        mod.set_axon_ntff_profile_hook = _set
        sys.modules["antenv.axon_hooks"] = mod
        antenv.axon_hooks = mod
    except Exception:
        pass


# ---------------------------------------------------------------------------
B, S, HID = 2, 2048, 2048
NH, G = 32, 4
HD = 64          # head dim
HKV = 8          # kv heads
THETA = 10000.0
P = 128          # partitions
NKT = HID // P   # 16 k-tiles over the contraction dim
NST = S // P     # 16 s-tiles
NCH = 4          # s-chunks of 512
CH = 512
DV = 512         # v columns per core (8 heads x 64)
VROW = 528       # v tile row: [256 v | 1 one | 7 pad] x 2 heads
VOFF = 264

F32 = mybir.dt.float32
F16 = mybir.dt.float16

_CACHE = {}
LAST_RESULTS = None


def _build():
    nc = bass.Bass("TRN2")
    d_xt = nc.declare_dram_parameter("xt", [HID, S], F16, isOutput=False)
    d_wq = nc.declare_dram_parameter("wq", [HID, P], F16, isOutput=False)
    d_wk = nc.declare_dram_parameter("wk", [HID, P], F16, isOutput=False)
    d_wv = nc.declare_dram_parameter("wv", [HID, DV], F16, isOutput=False)
    d_cos = nc.declare_dram_parameter("cost", [P, S], F16, isOutput=False)
    d_sin = nc.declare_dram_parameter("sint", [P, S], F16, isOutput=False)
    d_tri = nc.declare_dram_parameter("tri", [P, P], F16, isOutput=False)
    d_out = nc.declare_dram_parameter("out", [S, DV], F16, isOutput=True)

    with tile.TileContext(nc) as tc, ExitStack() as ctx:
        pers = ctx.enter_context(tc.tile_pool(name="pers", bufs=1))
        epool = ctx.enter_context(tc.tile_pool(name="epool", bufs=64))
        outp = ctx.enter_context(tc.tile_pool(name="outp", bufs=3))
        psum = ctx.enter_context(tc.tile_pool(name="psum", bufs=8, space="PSUM"))

        def ps_tile(name):
            return psum.tile([P, CH], F32, tag="ps", bufs=8, name=name)

        # --- persistent tiles; DMA order tuned so PE never starves:
        # wq + xt0 first (first MM at ~2.8us), wk, cos/sin early (rope needs
        # them at ~31us), then the xt stream with per-k wv slices interleaved
        # (wv fully lands just before the V projection starts).
        wq_sb = pers.tile([P, NKT, P], F16, tag="wq")
        wk_sb = pers.tile([P, NKT, P], F16, tag="wk")
        wv_sb = pers.tile([P, NKT, DV], F16, tag="wv")
        cos_sb = pers.tile([P, S], F16, tag="cos")
        sin_sb = pers.tile([P, S], F16, tag="sin")
        tri_sb = pers.tile([P, P], F16, tag="tri")
        xt = [pers.tile([P, S], F16, tag=f"xt{k}", name=f"xt{k}") for k in range(NKT)]

        # Input stream split across BOTH hardware DGE queues (sync + scalar):
        # a single queue serializes on ~0.6-1.3us issue cost per DMA plus
        # semaphore recycling and stretches delivery to ~43us; two queues
        # roughly halve that. ACT is idle during phase 1 so its queue is free.
        # two hardware-DGE streams (sync + scalar), interleaved xt/wv. Do NOT
        # put bulk input on gpsimd's SWDGE: it starves the HWDGE xt stream.
        nc.sync.dma_start(out=wq_sb[:], in_=d_wq[:].rearrange("(kt p) m -> p kt m", p=P))
        nc.scalar.dma_start(out=wk_sb[:], in_=d_wk[:].rearrange("(kt p) m -> p kt m", p=P))
        nc.sync.dma_start(out=xt[0][:], in_=d_xt[bass.ts(0, P), :])
        nc.scalar.dma_start(out=xt[1][:], in_=d_xt[bass.ts(1, P), :])
        nc.sync.dma_start(out=cos_sb[:], in_=d_cos[:])
        nc.scalar.dma_start(out=sin_sb[:], in_=d_sin[:])
        for k in range(2, NKT):
            eng = nc.sync if k % 2 == 0 else nc.scalar
            eng.dma_start(out=xt[k][:], in_=d_xt[bass.ts(k, P), :])
            eng.dma_start(out=wv_sb[:, k - 2, :], in_=d_wv[bass.ts(k - 2, P), :])
        nc.sync.dma_start(out=wv_sb[:, NKT - 2, :], in_=d_wv[bass.ts(NKT - 2, P), :])
        nc.scalar.dma_start(out=wv_sb[:, NKT - 1, :], in_=d_wv[bass.ts(NKT - 1, P), :])
        nc.sync.dma_start(out=tri_sb[:], in_=d_tri[:])

        # qt/kt are built IN PLACE: ACT/DVE evacuate the projection psums into
        # qt/kt, a DMA shuffle builds the rotate-half copy, then two in-place
        # muls + one in-place add finish RoPE with no extra tiles.
        qt = pers.tile([P, S], F16, tag="qt")
        kt = pers.tile([P, S], F16, tag="kt")
        q_rot = pers.tile([P, S], F16, tag="q_rot")
        k_rot = pers.tile([P, S], F16, tag="k_rot")
        v_sb = [pers.tile([P, VROW], F16, tag=f"v{t}", name=f"v{t}") for t in range(NST)]

        # --- warmup: keep the PE busy during the DMA lead-in so the HAM
        # clock-gate releases (1.2 -> 2.4 GHz) before the real matmuls start.
        warm_sb = pers.tile([P, CH], F16, tag="warm")
        nc.vector.memset(warm_sb[:], 0.0)
        warm_ps = ps_tile("warm")
        for i in range(6):
            nc.tensor.matmul(warm_ps[:], lhsT=warm_sb[:, 0:P], rhs=warm_sb[:],
                             start=True, stop=True, skip_group_check=True)

        # ones columns of the V tiles (denominator trick), emitted early
        for t in range(NST):
            nc.vector.memset(v_sb[t][:, 256:257], 1.0)
            nc.vector.memset(v_sb[t][:, VOFF + 256:VOFF + 257], 1.0)

        # --- phase 1: Q and K projections interleaved per xt tile (8 MMs per
        # DMA arrival keeps PE duty high while the xt stream lands).
        pq = [ps_tile(f"pq{c}") for c in range(NCH)]
        pk = [ps_tile(f"pk{c}") for c in range(NCH)]
        for k in range(NKT):
            for c in range(NCH):
                nc.tensor.matmul(pq[c][:], lhsT=wq_sb[:, k, :], rhs=xt[k][:, bass.ts(c, CH)],
                                 start=(k == 0), stop=(k == NKT - 1), skip_group_check=True)
            for c in range(NCH):
                nc.tensor.matmul(pk[c][:], lhsT=wk_sb[:, k, :], rhs=xt[k][:, bass.ts(c, CH)],
                                 start=(k == 0), stop=(k == NKT - 1), skip_group_check=True)

        # --- evacuate Q/K psums fast (frees banks for the V projection):
        # ACT takes Q, DVE takes K.
        for c in range(NCH):
            nc.scalar.copy(qt[:, bass.ts(c, CH)], pq[c][:])
        for c in range(NCH):
            nc.vector.tensor_copy(out=kt[:, bass.ts(c, CH)], in_=pk[c][:])

        # rotate-half via SBUF->SBUF DMA (off every compute engine): block b of
        # the rot tile is block b^1 of the raw tile; the sign lives in the sin
        # table. Then RoPE is 3 full-width partition-aligned in-place fp16 DVE
        # ops per tensor (walrus rejects partition-shifted tensor_tensor when
        # both inputs are SBUF, so the shift must happen in a copy).
        # shuffle DMAs go on gpsimd's SWDGE queue so they do not sit behind
        # the tail of the input stream on the hardware DGE queues
        for blk in range(4):
            lo, swap_lo = blk * 32, (blk ^ 1) * 32
            nc.gpsimd.dma_start(out=q_rot[lo:lo + 32, :], in_=qt[swap_lo:swap_lo + 32, :])
            nc.gpsimd.dma_start(out=k_rot[lo:lo + 32, :], in_=kt[swap_lo:swap_lo + 32, :])

        def rope(raw, rot, cs):
            nc.vector.tensor_mul(out=raw[:, cs], in0=raw[:, cs], in1=cos_sb[:, cs])
            nc.vector.tensor_mul(out=rot[:, cs], in0=rot[:, cs], in1=sin_sb[:, cs])
            nc.vector.tensor_add(out=raw[:, cs], in0=raw[:, cs], in1=rot[:, cs])

        # chunk 0 of q AND k first: the first scores pair only needs cols
        # 0:512 of both, so it unblocks ~2us earlier than a full-width rope
        rope(qt, q_rot, bass.ds(0, CH))
        rope(kt, k_rot, bass.ds(0, CH))
        rope(qt, q_rot, bass.ds(CH, S - CH))
        rope(kt, k_rot, bass.ds(CH, S - CH))

        # --- attention work generators (pumped between V-proj s-tiles) ---
        e_tiles = {}

        def gen_scores(c):
            cs0 = c * CH
            nk = 4 * c + 4
            tiles = [[None] * nk for _ in range(2)]
            e_tiles[c] = tiles
            for t in range(nk):
                m = t - 4 * c
                off = max(m, 0) * P
                w = CH - off
                for h in range(2):
                    ps_s = ps_tile("ps_s")
                    nc.tensor.matmul(
                        ps_s[:, 0:w],
                        lhsT=kt[h * HD:(h + 1) * HD, bass.ts(t, P)],
                        rhs=qt[h * HD:(h + 1) * HD, bass.ds(cs0 + off, w)],
                        start=True, stop=True, skip_group_check=True)
                    e = epool.tile([P, CH], F16, tag="e", name=f"e{h}_{t}")
                    nc.scalar.activation(e[:, bass.ds(off, w)], ps_s[:, 0:w],
                                         mybir.ActivationFunctionType.Exp)
                    if m >= 0:
                        nc.vector.tensor_mul(out=e[:, bass.ts(m, P)],
                                             in0=e[:, bass.ts(m, P)], in1=tri_sb[:])
                    tiles[h][t] = e
                yield 2  # 2 MMs emitted

        def gen_av(c):
            tiles = e_tiles.pop(c)
            for m in range(4):
                q_idx = 4 * c + m
                out_stage = outp.tile([P, DV], F16, tag="out_stage", name="out_stage")
                for h in range(2):
                    po = ps_tile("po")
                    for t in range(q_idx + 1):
                        nc.tensor.matmul(
                            po[:, 0:257],
                            lhsT=tiles[h][t][:, bass.ts(m, P)],
                            rhs=v_sb[t][:, h * VOFF:h * VOFF + 257],
                            start=(t == 0), stop=(t == q_idx), skip_group_check=True)
                    rec = outp.tile([P, 1], F32, tag="rec", name="rec")
                    nc.vector.reciprocal(rec[:], po[:, 256:257])
                    nc.vector.tensor_scalar_mul(
                        out_stage[:, bass.ts(h, 256)], po[:, 0:256], rec[:])
                    # per-half DMA: h0's store overlaps h1's matmuls (matters
                    # for the very last q tile, which is on the critical path)
                    nc.sync.dma_start(out=d_out[bass.ts(q_idx, P), bass.ts(h, 256)],
                                      in_=out_stage[:, bass.ts(h, 256)])
                    yield q_idx + 1

        # queue of (gate_tile, generator): emission-order gates tuned so no PE
        # instruction ever waits on rope (scores c needs rope; av c needs
        # v_sb[4c+3] and E tiles).
        # NOTE on epool sizing: 'e' allocations total 80; with 64 buffers the
        # last 16 allocations (scores(3) t>=8) wrap onto the buffers of c0/c1,
        # whose last readers (av(0)/av(1)) sit EARLIER in the PE queue -- the
        # WAR dependency points backward, so no deadlock.
        #
        # The pump round-robins across all gated-ready generators and caps
        # scores yields per call: each scores MM costs ACT a 730ns exp but PE
        # only 216ns, so long contiguous scores runs outpace ACT, stall the PE
        # on psum-bank recycling, and drop the HAM clock to 1.2 GHz.
        scores_done = {}
        work_items = [
            ["s", 0, 3, gen_scores(0), False],
            ["s", 1, 4, gen_scores(1), False],
            ["av", 0, 4, gen_av(0), False],
            ["s", 2, 5, gen_scores(2), False],
            ["av", 1, 7, gen_av(1), False],
            ["s", 3, 6, gen_scores(3), False],
            ["av", 2, 11, gen_av(2), False],
            ["av", 3, 15, gen_av(3), False],
        ]

        def pump(t_done, budget, scap):
            emitted = 0
            sc = 0
            progress = True
            while progress and emitted < budget:
                progress = False
                for item in work_items:
                    kind, cid, gate, gen, done = item
                    if done or gate > t_done:
                        continue
                    if kind == "s" and sc >= scap:
                        continue
                    if kind == "av" and not scores_done.get(cid):
                        continue
                    try:
                        emitted += next(gen)
                        if kind == "s":
                            sc += 1
                        progress = True
                    except StopIteration:
                        item[4] = True
                        if kind == "s":
                            scores_done[cid] = True
                    if emitted >= budget:
                        break
            return emitted

        # --- phase 2: V projection with attention work interleaved ---
        for t in range(NST):
            pv = ps_tile("pv")
            for k in range(NKT):
                nc.tensor.matmul(pv[:], lhsT=xt[k][:, bass.ts(t, P)], rhs=wv_sb[:, k, :],
                                 start=(k == 0), stop=(k == NKT - 1), skip_group_check=True)
            # V evac: early rounds on ACT (exp has not started; DVE must get to
            # rope immediately), later rounds on DVE (ACT saturated by exp)
            if t < 6:
                nc.scalar.copy(v_sb[t][:, 0:256], pv[:, 0:256])
                nc.scalar.copy(v_sb[t][:, VOFF:VOFF + 256], pv[:, 256:512])
            else:
                nc.vector.tensor_copy(out=v_sb[t][:, 0:256], in_=pv[:, 0:256])
                nc.vector.tensor_copy(out=v_sb[t][:, VOFF:VOFF + 256], in_=pv[:, 256:512])
            pump(t, 40, 4)
        while not all(item[4] for item in work_items):
            pump(NST, 10 ** 9, 10 ** 9)

    _legalize_waits(nc)
    _dedup_ldweights(nc)
    return nc


def _host_prep(hidden_states, position_ids, Wq, Wk, Wv):
    """Build the 8 per-core input maps."""
    hidden_states = np.asarray(hidden_states, dtype=np.float32)
    position_ids = np.asarray(position_ids)
    Wq = np.asarray(Wq, dtype=np.float32)
    Wk = np.asarray(Wk, dtype=np.float32)
    Wv = np.asarray(Wv, dtype=np.float32)

    scale = 1.0 / np.sqrt(HD)
    tri = np.triu(np.ones((P, P), dtype=np.float32)).astype(np.float16)
    inv_freq = (1.0 / (THETA ** (np.arange(0, HD, 2, dtype=np.float32) / HD))).astype(np.float32)

    in_maps = []
    for c in range(8):
        b, p = c // 4, c % 4
        xt = np.ascontiguousarray(hidden_states[b].T).astype(np.float16)
        wq = (Wq[:, p * P:(p + 1) * P] * scale).astype(np.float16)
        wk = Wk[:, p * P:(p + 1) * P].astype(np.float16)
        cols = []
        for h in (2 * p, 2 * p + 1):
            for r in range(G):
                j = r * HKV + h
                cols.append(Wv[:, j * HD:(j + 1) * HD])
        wv = np.concatenate(cols, axis=1).astype(np.float16)

        pos = position_ids[b].astype(np.float32)
        freqs = pos[:, None] * inv_freq[None, :]          # [S, 32]
        cos32 = np.cos(freqs).T.astype(np.float32)        # [32, S]
        sin32 = np.sin(freqs).T.astype(np.float32)
        cost = np.ascontiguousarray(np.tile(cos32, (4, 1))).astype(np.float16)  # [128, S]
        sint = np.ascontiguousarray(
            np.concatenate([-sin32, sin32, -sin32, sin32], axis=0)).astype(np.float16)

        in_maps.append({
            "xt": xt, "wq": wq, "wk": wk, "wv": wv,
            "cost": cost, "sint": sint, "tri": tri,
        })
    return in_maps


def kernel(hidden_states, position_ids, Wq, Wk, Wv):
    global LAST_RESULTS
    trace = bool(os.environ.get("CHEEMS_TRACE"))
    if trace:
        _install_ntff_hook()
    if "nc" not in _CACHE:
        _CACHE["nc"] = _build()
    nc = _CACHE["nc"]
    in_maps = _host_prep(hidden_states, position_ids, Wq, Wk, Wv)
    res = run_bass_kernel_spmd(nc, in_maps, core_ids=list(range(8)), trace=trace)
    LAST_RESULTS = res

    out = np.empty((B, S, HID), dtype=np.float32)
    for c in range(8):
        b, p = c // 4, c % 4
        core_out = np.asarray(res.results[c]["out"], dtype=np.float32)  # [S, 512]
        for hl, h in enumerate((2 * p, 2 * p + 1)):
            for r in range(G):
                j = r * HKV + h
                out[b, :, j * HD:(j + 1) * HD] = core_out[:, (hl * G + r) * HD:(hl * G + r + 1) * HD]
    return out.reshape(B, S, HID)


# revision 23
# speedup vs baseline: 1.1354x; 1.0126x over previous
"""Trainium2 Bass kernel for CheemsNonWoAttn (GQA attention block, no Wo).

Sharding: 8 cores = batch(2) x kv-head-pair(4). Each core handles one batch
element and 2 of the 8 kv heads (GQA: Q and K are repeated identically across
the 4 groups, so only 8 unique softmax matrices exist; V uses all 32 heads).

Per-core device program (v2, stall-free schedule):
  warmup MMs during DMA lead-in (HAM clock-gate release)
  Qt/Kt = Wq^T X^T, Wk^T X^T    (d on partitions; N=512 matmuls, K accum=16)
  psum evac split: ACT copies Q psums, DVE copies K psums -> banks free fast
  RoPE on DVE in fp16 (2x rate), 6 ops per 1024-col pair, off critical path
  V = X Wv interleaved with attention work (pump), scores gated so the
  in-order PE queue never waits on rope
  scores^T[k, q] = Kt^T Qt      (per head, K=64)
  E = exp(scores) on ACT        (no max subtraction: |scores| <~ 6, safe)
  causal mask: multiply 128x128 diagonal blocks by triangular mask (DVE)
  out = E^T V with a ones-column in V producing the softmax denominator
  out /= denom (DVE reciprocal + tensor_scalar), fp16 out, host upconverts
"""
import os
import sys
import types
from contextlib import ExitStack

for _p in ("/opt/trn_rl_repo", "/root/.axon_site/_ro/trn_rl_repo"):
    if os.path.isdir(_p) and _p not in sys.path:
        sys.path.append(_p)

import numpy as np

import concourse.bass as bass
import concourse.tile as tile
from concourse import mybir
from concourse.bass_utils import run_bass_kernel_spmd
from concourse.vector_clock import ScopedClock

# ---------------------------------------------------------------------------
# Patch 1: walrus rejects Drain instructions with >1 sync wait (CTRL ops have
# a single wait slot). Split the TileContext exit drain's waits across extra
# SP nops, one wait each.
def _patched_drain_and_barrier(self, tick_clock, wait_clock):
    nc = self.nc
    drain_bi = nc.sync.drain()
    wait_clock.add_sem_waits(drain_bi.ins, ScopedClock({None: tick_clock.global_clock}))
    inst = drain_bi.ins
    si = inst.sync_info
    if si is not None and si.on_wait is not None and len(si.on_wait) > 1:
        waits = list(si.on_wait)
        inst.sync_info = mybir.SyncInfo(
            on_wait=waits[:1],
            on_update=list(si.on_update) if si.on_update else [],
        )
        for w in waits[1:]:
            nbi = nc.sync.nop()
            nbi.ins.sync_info = mybir.SyncInfo(on_wait=[w], on_update=[])
    nc.all_engine_barrier()
    assert self.sems is not None
    popped = nc._tile_sem_poison_stack.pop()
    assert popped is self._sem_poison
    nc.clear_and_free_semaphores(list(self.sems.allocated().values()))
    if not os.environ.get("CHEEMS_TRIM_EXIT_BARRIER"):
        nc.all_engine_barrier()


tile.TileContext._drain_and_barrier = _patched_drain_and_barrier


def _legalize_waits(nc):
    """This walrus build accepts at most one sync-wait per instruction.
    Split any instruction carrying N>1 waits into N-1 preceding same-engine
    nops (engines are in-order, so semantics are preserved)."""
    uid = 0
    for f in nc.m.functions:
        for blk in f.blocks:
            insts = list(blk.instructions)
            out, changed = [], False
            for inst in insts:
                si = getattr(inst, "sync_info", None)
                if si is not None and si.on_wait is not None and len(si.on_wait) > 1:
                    waits = list(si.on_wait)
                    for w in waits[:-1]:
                        uid += 1
                        out.append(mybir.InstNoOp(
                            name=f"{inst.name}_lw{uid}",
                            engine=inst.engine,
                            sync_info=mybir.SyncInfo(on_wait=[w], on_update=[]),
                            bass_nofuse=True,
                        ))
                    inst.sync_info = mybir.SyncInfo(
                        on_wait=waits[-1:],
                        on_update=list(si.on_update) if si.on_update else [],
                    )
                    changed = True
                out.append(inst)
            if changed:
                blk.instructions = out


def _dedup_ldweights(nc):
    """Clear the self-load on matmuls whose stationary operand AP is identical
    to the immediately preceding matmul in the final PE stream (walrus runs
    with ldw-opt disabled, so it reloads weights for every matmul otherwise)."""
    if os.environ.get("CHEEMS_NO_LDW_DEDUP"):
        return
    for f in nc.m.functions:
        for blk in f.blocks:
            prev_key = None
            for inst in blk.instructions:
                if not isinstance(inst, mybir.InstMatmult):
                    if isinstance(inst, mybir.InstLdweights):
                        prev_key = None
                    continue
                key = (repr(inst.ins[1]), inst.perf_mode, inst.is_transpose,
                       repr(inst.tile_position))
                if prev_key is not None and key == prev_key:
                    inst.ldweights = False
                prev_key = key


# Patch 2 (optional, for tracing): recreate the antenv.axon_hooks shim so
# run_bass_kernel_spmd(trace=True) can capture NTFF profiles under axon.
def _install_ntff_hook():
    try:
        if "antenv.axon_hooks" in sys.modules:
            return
        import antenv
        from trn_agent_boot.trn_boot import _ntff_profile_via_ctypes

        hook = _ntff_profile_via_ctypes("/opt/axon/libaxon_pjrt.so")
        mod = types.ModuleType("antenv.axon_hooks")
        mod._hook = hook
        mod.get_axon_ntff_profile_hook = lambda: mod._hook

        def _set(h):
            mod._hook = h

        mod.set_axon_ntff_profile_hook = _set
        sys.modules["antenv.axon_hooks"] = mod
        antenv.axon_hooks = mod
    except Exception:
        pass


# ---------------------------------------------------------------------------
B, S, HID = 2, 2048, 2048
NH, G = 32, 4
HD = 64          # head dim
HKV = 8          # kv heads
THETA = 10000.0
P = 128          # partitions
NKT = HID // P   # 16 k-tiles over the contraction dim
NST = S // P     # 16 s-tiles
NCH = 4          # s-chunks of 512
CH = 512
DV = 512         # v columns per core (8 heads x 64)
VROW = 528       # v tile row: [256 v | 1 one | 7 pad] x 2 heads
VOFF = 264

F32 = mybir.dt.float32
F16 = mybir.dt.float16

_CACHE = {}
LAST_RESULTS = None


def _build():
    nc = bass.Bass("TRN2")
    d_xt = nc.declare_dram_parameter("xt", [HID, S], F16, isOutput=False)
    d_wq = nc.declare_dram_parameter("wq", [HID, P], F16, isOutput=False)
    d_wk = nc.declare_dram_parameter("wk", [HID, P], F16, isOutput=False)
    d_wv = nc.declare_dram_parameter("wv", [HID, DV], F16, isOutput=False)
    d_cos = nc.declare_dram_parameter("cost", [P, S], F16, isOutput=False)
    d_sin = nc.declare_dram_parameter("sint", [P, S], F16, isOutput=False)
    d_tri = nc.declare_dram_parameter("tri", [P, P], F16, isOutput=False)
    d_out = nc.declare_dram_parameter("out", [S, DV], F16, isOutput=True)

    with tile.TileContext(nc) as tc, ExitStack() as ctx:
        pers = ctx.enter_context(tc.tile_pool(name="pers", bufs=1))
        epool = ctx.enter_context(tc.tile_pool(name="epool", bufs=64))
        outp = ctx.enter_context(tc.tile_pool(name="outp", bufs=3))
        psum = ctx.enter_context(tc.tile_pool(name="psum", bufs=8, space="PSUM"))

        def ps_tile(name):
            return psum.tile([P, CH], F32, tag="ps", bufs=8, name=name)

        # --- persistent tiles; DMA order tuned so PE never starves:
        # wq + xt0 first (first MM at ~2.8us), wk, cos/sin early (rope needs
        # them at ~31us), then the xt stream with per-k wv slices interleaved
        # (wv fully lands just before the V projection starts).
        wq_sb = pers.tile([P, NKT, P], F16, tag="wq")
        wk_sb = pers.tile([P, NKT, P], F16, tag="wk")
        wv_sb = pers.tile([P, NKT, DV], F16, tag="wv")
        cos_sb = pers.tile([P, S], F16, tag="cos")
        sin_sb = pers.tile([P, S], F16, tag="sin")
        tri_sb = pers.tile([P, P], F16, tag="tri")
        xt = [pers.tile([P, S], F16, tag=f"xt{k}", name=f"xt{k}") for k in range(NKT)]

        # Input stream split across BOTH hardware DGE queues (sync + scalar):
        # a single queue serializes on ~0.6-1.3us issue cost per DMA plus
        # semaphore recycling and stretches delivery to ~43us; two queues
        # roughly halve that. ACT is idle during phase 1 so its queue is free.
        # two hardware-DGE streams (sync + scalar), interleaved xt/wv. Do NOT
        # put bulk input on gpsimd's SWDGE: it starves the HWDGE xt stream.
        # phase 1 only consumes xt, so the xt stream gets the bandwidth first;
        # cos/sin slot in mid-stream (needed at rope, ~30us), wv and tri trail
        # (needed at the V projection / first exp, also ~30us).
        nc.sync.dma_start(out=wq_sb[:], in_=d_wq[:].rearrange("(kt p) m -> p kt m", p=P))
        nc.scalar.dma_start(out=wk_sb[:], in_=d_wk[:].rearrange("(kt p) m -> p kt m", p=P))
        for k in range(NKT):
            eng = nc.sync if k % 2 == 0 else nc.scalar
            eng.dma_start(out=xt[k][:], in_=d_xt[bass.ts(k, P), :])
            if k == 8:
                nc.sync.dma_start(out=cos_sb[:], in_=d_cos[:])
                nc.scalar.dma_start(out=sin_sb[:], in_=d_sin[:])
        nc.sync.dma_start(out=tri_sb[:], in_=d_tri[:])
        for k in range(NKT):
            eng = nc.sync if k % 2 == 0 else nc.scalar
            eng.dma_start(out=wv_sb[:, k, :], in_=d_wv[bass.ts(k, P), :])

        # qt/kt are built IN PLACE: ACT/DVE evacuate the projection psums into
        # qt/kt, a DMA shuffle builds the rotate-half copy, then two in-place
        # muls + one in-place add finish RoPE with no extra tiles.
        qt = pers.tile([P, S], F16, tag="qt")
        kt = pers.tile([P, S], F16, tag="kt")
        q_rot = pers.tile([P, S], F16, tag="q_rot")
        k_rot = pers.tile([P, S], F16, tag="k_rot")
        v_sb = [pers.tile([P, VROW], F16, tag=f"v{t}", name=f"v{t}") for t in range(NST)]

        # --- warmup: keep the PE busy during the DMA lead-in so the HAM
        # clock-gate releases (1.2 -> 2.4 GHz) before the real matmuls start.
        warm_sb = pers.tile([P, CH], F16, tag="warm")
        nc.vector.memset(warm_sb[:], 0.0)
        warm_ps = ps_tile("warm")
        for i in range(6):
            nc.tensor.matmul(warm_ps[:], lhsT=warm_sb[:, 0:P], rhs=warm_sb[:],
                             start=True, stop=True, skip_group_check=True)

        # ones columns of the V tiles (denominator trick), emitted early
        for t in range(NST):
            nc.vector.memset(v_sb[t][:, 256:257], 1.0)
            nc.vector.memset(v_sb[t][:, VOFF + 256:VOFF + 257], 1.0)

        # --- phase 1: Q and K projections interleaved per xt tile (8 MMs per
        # DMA arrival keeps PE duty high while the xt stream lands).
        pq = [ps_tile(f"pq{c}") for c in range(NCH)]
        pk = [ps_tile(f"pk{c}") for c in range(NCH)]
        for k in range(NKT):
            for c in range(NCH):
                nc.tensor.matmul(pq[c][:], lhsT=wq_sb[:, k, :], rhs=xt[k][:, bass.ts(c, CH)],
                                 start=(k == 0), stop=(k == NKT - 1), skip_group_check=True)
            for c in range(NCH):
                nc.tensor.matmul(pk[c][:], lhsT=wk_sb[:, k, :], rhs=xt[k][:, bass.ts(c, CH)],
                                 start=(k == 0), stop=(k == NKT - 1), skip_group_check=True)

        # --- evacuate Q/K psums fast (frees banks for the V projection):
        # ACT takes Q, DVE takes K.
        for c in range(NCH):
            nc.scalar.copy(qt[:, bass.ts(c, CH)], pq[c][:])
        for c in range(NCH):
            nc.vector.tensor_copy(out=kt[:, bass.ts(c, CH)], in_=pk[c][:])

        # rotate-half via SBUF->SBUF DMA (off every compute engine): block b of
        # the rot tile is block b^1 of the raw tile; the sign lives in the sin
        # table. Then RoPE is 3 full-width partition-aligned in-place fp16 DVE
        # ops per tensor (walrus rejects partition-shifted tensor_tensor when
        # both inputs are SBUF, so the shift must happen in a copy).
        # shuffle DMAs go on gpsimd's SWDGE queue so they do not sit behind
        # the tail of the input stream on the hardware DGE queues
        for blk in range(4):
            lo, swap_lo = blk * 32, (blk ^ 1) * 32
            nc.gpsimd.dma_start(out=q_rot[lo:lo + 32, :], in_=qt[swap_lo:swap_lo + 32, :])
            nc.gpsimd.dma_start(out=k_rot[lo:lo + 32, :], in_=kt[swap_lo:swap_lo + 32, :])

        def rope(raw, rot, cs):
            nc.vector.tensor_mul(out=raw[:, cs], in0=raw[:, cs], in1=cos_sb[:, cs])
            nc.vector.tensor_mul(out=rot[:, cs], in0=rot[:, cs], in1=sin_sb[:, cs])
            nc.vector.tensor_add(out=raw[:, cs], in0=raw[:, cs], in1=rot[:, cs])

        # chunk 0 of q AND k first: the first scores pair only needs cols
        # 0:512 of both, so it unblocks ~2us earlier than a full-width rope
        rope(qt, q_rot, bass.ds(0, CH))
        rope(kt, k_rot, bass.ds(0, CH))
        rope(qt, q_rot, bass.ds(CH, S - CH))
        rope(kt, k_rot, bass.ds(CH, S - CH))

        # --- attention work generators (pumped between V-proj s-tiles) ---
        e_tiles = {}

        def gen_scores(c):
            cs0 = c * CH
            nk = 4 * c + 4
            tiles = [[None] * nk for _ in range(2)]
            e_tiles[c] = tiles
            for t in range(nk):
                m = t - 4 * c
                off = max(m, 0) * P
                w = CH - off
                for h in range(2):
                    ps_s = ps_tile("ps_s")
                    nc.tensor.matmul(
                        ps_s[:, 0:w],
                        lhsT=kt[h * HD:(h + 1) * HD, bass.ts(t, P)],
                        rhs=qt[h * HD:(h + 1) * HD, bass.ds(cs0 + off, w)],
                        start=True, stop=True, skip_group_check=True)
                    e = epool.tile([P, CH], F16, tag="e", name=f"e{h}_{t}")
                    nc.scalar.activation(e[:, bass.ds(off, w)], ps_s[:, 0:w],
                                         mybir.ActivationFunctionType.Exp)
                    if m >= 0:
                        nc.vector.tensor_mul(out=e[:, bass.ts(m, P)],
                                             in0=e[:, bass.ts(m, P)], in1=tri_sb[:])
                    tiles[h][t] = e
                yield 2  # 2 MMs emitted

        def gen_av(c):
            tiles = e_tiles.pop(c)
            for m in range(4):
                q_idx = 4 * c + m
                out_stage = outp.tile([P, DV], F16, tag="out_stage", name="out_stage")
                for h in range(2):
                    po = ps_tile("po")
                    for t in range(q_idx + 1):
                        nc.tensor.matmul(
                            po[:, 0:257],
                            lhsT=tiles[h][t][:, bass.ts(m, P)],
                            rhs=v_sb[t][:, h * VOFF:h * VOFF + 257],
                            start=(t == 0), stop=(t == q_idx), skip_group_check=True)
                    rec = outp.tile([P, 1], F32, tag="rec", name="rec")
                    nc.vector.reciprocal(rec[:], po[:, 256:257])
                    nc.vector.tensor_scalar_mul(
                        out_stage[:, bass.ts(h, 256)], po[:, 0:256], rec[:])
                    # per-half DMA: h0's store overlaps h1's matmuls (matters
                    # for the very last q tile, which is on the critical path)
                    nc.sync.dma_start(out=d_out[bass.ts(q_idx, P), bass.ts(h, 256)],
                                      in_=out_stage[:, bass.ts(h, 256)])
                    yield q_idx + 1

        # queue of (gate_tile, generator): emission-order gates tuned so no PE
        # instruction ever waits on rope (scores c needs rope; av c needs
        # v_sb[4c+3] and E tiles).
        # NOTE on epool sizing: 'e' allocations total 80; with 64 buffers the
        # last 16 allocations (scores(3) t>=8) wrap onto the buffers of c0/c1,
        # whose last readers (av(0)/av(1)) sit EARLIER in the PE queue -- the
        # WAR dependency points backward, so no deadlock.
        #
        # The pump round-robins across all gated-ready generators and caps
        # scores yields per call: each scores MM costs ACT a 730ns exp but PE
        # only 216ns, so long contiguous scores runs outpace ACT, stall the PE
        # on psum-bank recycling, and drop the HAM clock to 1.2 GHz.
        scores_done = {}
        work_items = [
            ["s", 0, 3, gen_scores(0), False],
            ["s", 1, 4, gen_scores(1), False],
            ["av", 0, 4, gen_av(0), False],
            ["s", 2, 5, gen_scores(2), False],
            ["av", 1, 7, gen_av(1), False],
            ["s", 3, 6, gen_scores(3), False],
            ["av", 2, 11, gen_av(2), False],
            ["av", 3, 15, gen_av(3), False],
        ]

        def pump(t_done, budget, scap):
            emitted = 0
            sc = 0
            progress = True
            while progress and emitted < budget:
                progress = False
                for item in work_items:
                    kind, cid, gate, gen, done = item
                    if done or gate > t_done:
                        continue
                    if kind == "s" and sc >= scap:
                        continue
                    if kind == "av" and not scores_done.get(cid):
                        continue
                    try:
                        emitted += next(gen)
                        if kind == "s":
                            sc += 1
                        progress = True
                    except StopIteration:
                        item[4] = True
                        if kind == "s":
                            scores_done[cid] = True
                    if emitted >= budget:
                        break
            return emitted

        # --- phase 2: V projection with attention work interleaved ---
        for t in range(NST):
            pv = ps_tile("pv")
            for k in range(NKT):
                nc.tensor.matmul(pv[:], lhsT=xt[k][:, bass.ts(t, P)], rhs=wv_sb[:, k, :],
                                 start=(k == 0), stop=(k == NKT - 1), skip_group_check=True)
            # V evac: early rounds on ACT (exp has not started; DVE must get to
            # rope immediately), later rounds on DVE (ACT saturated by exp)
            if t < 6:
                nc.scalar.copy(v_sb[t][:, 0:256], pv[:, 0:256])
                nc.scalar.copy(v_sb[t][:, VOFF:VOFF + 256], pv[:, 256:512])
            else:
                nc.vector.tensor_copy(out=v_sb[t][:, 0:256], in_=pv[:, 0:256])
                nc.vector.tensor_copy(out=v_sb[t][:, VOFF:VOFF + 256], in_=pv[:, 256:512])
            pump(t, 40, 4)
        while not all(item[4] for item in work_items):
            pump(NST, 10 ** 9, 10 ** 9)

    _legalize_waits(nc)
    _dedup_ldweights(nc)
    return nc


def _host_prep(hidden_states, position_ids, Wq, Wk, Wv):
    """Build the 8 per-core input maps."""
    hidden_states = np.asarray(hidden_states, dtype=np.float32)
    position_ids = np.asarray(position_ids)
    Wq = np.asarray(Wq, dtype=np.float32)
    Wk = np.asarray(Wk, dtype=np.float32)
    Wv = np.asarray(Wv, dtype=np.float32)

    scale = 1.0 / np.sqrt(HD)
    tri = np.triu(np.ones((P, P), dtype=np.float32)).astype(np.float16)
    inv_freq = (1.0 / (THETA ** (np.arange(0, HD, 2, dtype=np.float32) / HD))).astype(np.float32)

    in_maps = []
    for c in range(8):
        b, p = c // 4, c % 4
        xt = np.ascontiguousarray(hidden_states[b].T).astype(np.float16)
        wq = (Wq[:, p * P:(p + 1) * P] * scale).astype(np.float16)
        wk = Wk[:, p * P:(p + 1) * P].astype(np.float16)
        cols = []
        for h in (2 * p, 2 * p + 1):
            for r in range(G):
                j = r * HKV + h
                cols.append(Wv[:, j * HD:(j + 1) * HD])
        wv = np.concatenate(cols, axis=1).astype(np.float16)

        pos = position_ids[b].astype(np.float32)
        freqs = pos[:, None] * inv_freq[None, :]          # [S, 32]
        cos32 = np.cos(freqs).T.astype(np.float32)        # [32, S]
        sin32 = np.sin(freqs).T.astype(np.float32)
        cost = np.ascontiguousarray(np.tile(cos32, (4, 1))).astype(np.float16)  # [128, S]
        sint = np.ascontiguousarray(
            np.concatenate([-sin32, sin32, -sin32, sin32], axis=0)).astype(np.float16)

        in_maps.append({
            "xt": xt, "wq": wq, "wk": wk, "wv": wv,
            "cost": cost, "sint": sint, "tri": tri,
        })
    return in_maps


def kernel(hidden_states, position_ids, Wq, Wk, Wv):
    global LAST_RESULTS
    trace = bool(os.environ.get("CHEEMS_TRACE"))
    if trace:
        _install_ntff_hook()
    if "nc" not in _CACHE:
        _CACHE["nc"] = _build()
    nc = _CACHE["nc"]
    in_maps = _host_prep(hidden_states, position_ids, Wq, Wk, Wv)
    res = run_bass_kernel_spmd(nc, in_maps, core_ids=list(range(8)), trace=trace)
    LAST_RESULTS = res

    out = np.empty((B, S, HID), dtype=np.float32)
    for c in range(8):
        b, p = c // 4, c % 4
        core_out = np.asarray(res.results[c]["out"], dtype=np.float32)  # [S, 512]
        for hl, h in enumerate((2 * p, 2 * p + 1)):
            for r in range(G):
                j = r * HKV + h
                out[b, :, j * HD:(j + 1) * HD] = core_out[:, (hl * G + r) * HD:(hl * G + r + 1) * HD]
    return out.reshape(B, S, HID)
